# revision 50
# baseline (speedup 1.0000x reference)
# Trainium2 Bass kernel for nn_DilatedAttention (B=2, L=4096, D=1024, H=16,
# dilation=2, window=256): pre-LN attention block + FFN with residuals.
#
# Sharding: 8 cores = 2 batches x 4 sequence chunks of 1024 tokens, each with a
# 256-token halo on the left for K/V. No collectives. Dilated attention
# decomposes into two independent parity strands; within a strand it is a
# causal sliding-window attention with window 128 (+self).
#
# Key implementation choices:
#  - Projections run in fp8e4m3 DoubleRow (256-deep contraction per pass)
#    except groups toggled to fp16 for accuracy (weights then stream in
#    slabs). fp8 weights are scaled by 64 on host (into e4m3 normal range)
#    and unscaled at PSUM drain.
#  - fp8 weights are SBUF-resident for the whole kernel, one DMA each.
#  - Attention computes S^T[k,q] directly (lhsT=kT block, rhs=qT window) into
#    one 2-bank [128,1024] PSUM strip per (strand, head): a single exp and a
#    single concatenated-mask multiply cover all five key blocks. Probs are
#    k-major so the ctx matmul needs no transposes; softmax denominators come
#    from an appended ones-column in the packed V tile; ctx accumulates via
#    windowed matmuls with per-element has_written (mixed accumulate/first-
#    write inside one instruction), and is normalized by a replicated
#    fast-approx reciprocal on the way to ctxT.
#  - o1 (x + attn_out) stays in SBUF f32: no DRAM round trip.
#  - Scalar-engine table phases stay contiguous (sqrt / exp / sqrt / gelu).
import sys

sys.path.insert(0, "/opt/trn_rl_repo")

import os
from contextlib import ExitStack

import numpy as np
import ml_dtypes

import concourse.bass as bass
import concourse.mybir as mybir
import concourse.tile as tile
from concourse import bacc
from concourse.bass import ds, ts

F32 = mybir.dt.float32
F16 = mybir.dt.float16
FP8 = mybir.dt.float8e4
AF = mybir.ActivationFunctionType
ALU = mybir.AluOpType
DR = mybir.MatmulPerfMode.DoubleRow

B, L, D, H, HD, HID = 2, 4096, 1024, 16, 64, 4096
P = 128
NCORES = 8
CHUNK = 1024          # own tokens per core
HALO = 256            # original-token halo
TL = 640              # strand length incl halo (128 + 512)
TOWN = 512            # own strand tokens per parity
NBD = D // P          # 8 d-blocks
NBH = HID // P        # 32 hidden blocks
NT = TL // P          # 5 strand token tiles
NTO = TOWN // P       # 4 own token tiles
EPS = 1e-5
SCALE = 1.0 / 8.0     # 1/sqrt(HD)
WS = 64.0             # fp8 weight scale (host multiplies, kernel divides)
IWS = 1.0 / WS

# S^T column offsets for the five key blocks (widths 128,256,256,256,128)
JOFF = [0, 128, 384, 640, 896]
JW = [128, 256, 256, 256, 128]
# S^T emission windows (col_off, width, kblock, q_off, bank_first): key block 2
# is split at the PSUM bank boundary so each bank has exactly one start=True
SWIN = [(0, 128, 0, 0, True), (128, 256, 1, 0, False),
        (384, 128, 2, 128, False), (512, 128, 2, 256, True),
        (640, 256, 3, 256, False), (896, 128, 4, 384, False)]

# fc1 runs in fp16 (streamed weight slabs): the all-fp8 configuration exceeds
# the 2e-2 accuracy gate (measured 2.34e-2); qkv+o+fc2 in fp8 with fc1 fp16
# measures 1.81e-2.
_cfg = os.environ.get("FP8CFG") or "1101"
FP8_QKV = _cfg[0] == "1"
FP8_O = _cfg[1] == "1"
FP8_F1 = _cfg[2] == "1"
FP8_F2 = _cfg[3] == "1"


def _emit(nc, has_bias):
    hbq, hbk, hbv, hbo, hb1, hb2 = has_bias
    dt_qkv = FP8 if FP8_QKV else F16
    dt_o = FP8 if FP8_O else F16
    dt_f1 = FP8 if FP8_F1 else F16
    dt_f2 = FP8 if FP8_F2 else F16

    xs = nc.dram_tensor("xs", [2, TL, D], F32, kind="ExternalInput").ap()
    wq = nc.dram_tensor("wq", [P, 4, 2, D], dt_qkv, kind="ExternalInput").ap()
    wk = nc.dram_tensor("wk", [P, 4, 2, D], dt_qkv, kind="ExternalInput").ap()
    wv = nc.dram_tensor("wv", [P, 4, 2, D], dt_qkv, kind="ExternalInput").ap()
    wo = nc.dram_tensor("wo", [P, 4, 2, D], dt_o, kind="ExternalInput").ap()
    w1 = nc.dram_tensor("w1", [P, 4, 2, HID], dt_f1, kind="ExternalInput").ap()
    w2 = nc.dram_tensor("w2", [P, 16, 2, D], dt_f2, kind="ExternalInput").ap()
    maskC = nc.dram_tensor("maskC", [P, 1024], F16, kind="ExternalInput").ap()
    ident = nc.dram_tensor("ident", [P, P], F16, kind="ExternalInput").ap()
    bias_in = {}
    if hbq:
        bias_in["bq"] = nc.dram_tensor("bq", [NBD, P], F32, kind="ExternalInput").ap()
    if hbk:
        bias_in["bk"] = nc.dram_tensor("bk", [NBD, P], F32, kind="ExternalInput").ap()
    if hbv:
        bias_in["bv"] = nc.dram_tensor("bv", [D], F32, kind="ExternalInput").ap()
    if hbo:
        bias_in["bo"] = nc.dram_tensor("bo", [D], F32, kind="ExternalInput").ap()
    if hb1:
        bias_in["b1"] = nc.dram_tensor("b1", [NBH, P], F32, kind="ExternalInput").ap()
    if hb2:
        bias_in["b2"] = nc.dram_tensor("b2", [D], F32, kind="ExternalInput").ap()
    ys = nc.dram_tensor("ys", [2, TOWN, D], F32, kind="ExternalOutput").ap()

    def bcast(ap1d, n):
        return bass.AP(tensor=ap1d.tensor, offset=ap1d.offset, ap=[[0, P], *ap1d.ap])

    def mm_win(ps_win, lhsT_of, rhs_of, ko, fp8, skip=False, first=True):
        """Accumulation group over ko 256-blocks (fp8 DR) or 2ko 128-blocks.
        `first`: this group is the first writer of its PSUM bank (start=True).
        Only ONE start=True is allowed per 2KB bank region — it clears the
        whole bank's has_written state (probe5)."""
        if fp8:
            for o2 in range(ko):
                nc.tensor.matmul(ps_win, lhsT=lhsT_of(o2), rhs=rhs_of(o2),
                                 start=(first and o2 == 0), stop=(o2 == ko - 1),
                                 perf_mode=DR, skip_group_check=skip)
        else:
            for o2 in range(ko):
                la, ra = lhsT_of(o2), rhs_of(o2)
                for kt in range(2):
                    nc.tensor.matmul(
                        ps_win, lhsT=la[:, kt], rhs=ra[:, kt],
                        start=(first and o2 == 0 and kt == 0),
                        stop=(o2 == ko - 1 and kt == 1), skip_group_check=skip)

    with tile.TileContext(nc) as tc:
        sConst = ExitStack()
        sW = ExitStack()
        sAB = ExitStack()    # z1T
        sBC = ExitStack()    # qT, kT, vv2
        sCD = ExitStack()    # ctxT
        sDF = ExitStack()    # o1

        cpool = sConst.enter_context(tc.tile_pool(name="const", bufs=1))
        mC = cpool.tile([P, 1024], F16, tag="mC")
        nc.sync.dma_start(mC, maskC)
        idt = cpool.tile([P, P], F16, tag="idt")
        nc.sync.dma_start(idt, ident)
        ones16 = cpool.tile([P, 64], F16, tag="ones16")
        nc.vector.memset(ones16, 1.0)
        eps_t = cpool.tile([P, 1], F32, tag="eps")
        nc.vector.memset(eps_t, EPS)
        bias_sb = {}
        if hbq:
            bias_sb["bq"] = cpool.tile([P, NBD], F32, tag="bq")
            nc.sync.dma_start(bias_sb["bq"], bias_in["bq"].rearrange("o p -> p o"))
        if hbk:
            bias_sb["bk"] = cpool.tile([P, NBD], F32, tag="bk")
            nc.sync.dma_start(bias_sb["bk"], bias_in["bk"].rearrange("o p -> p o"))
        if hbv:
            bias_sb["bv"] = cpool.tile([P, D], F32, tag="bv")
            nc.sync.dma_start(bias_sb["bv"], bcast(bias_in["bv"], D))
        if hbo:
            bias_sb["bo"] = cpool.tile([P, D], F32, tag="bo")
            nc.sync.dma_start(bias_sb["bo"], bcast(bias_in["bo"], D))
        if hb1:
            bias_sb["b1"] = cpool.tile([P, NBH], F32, tag="b1")
            nc.sync.dma_start(bias_sb["b1"], bias_in["b1"].rearrange("o p -> p o"))
        if hb2:
            bias_sb["b2"] = cpool.tile([P, D], F32, tag="b2")
            nc.sync.dma_start(bias_sb["b2"], bcast(bias_in["b2"], D))

        # ---- resident weights; DMAs issued after stage A's x loads ----
        wpool = sW.enter_context(tc.tile_pool(name="wts", bufs=1, side="right"))
        wq_sb = wpool.tile([P, 4, 2, D], dt_qkv, tag="wq_sb")
        wk_sb = wpool.tile([P, 4, 2, D], dt_qkv, tag="wk_sb")
        wv_sb = wpool.tile([P, 4, 2, D], dt_qkv, tag="wv_sb")
        wo_sb = wpool.tile([P, 4, 2, D], dt_o, tag="wo_sb")
        w1_sb = w2_sb = None
        if FP8_F1:
            w1_sb = wpool.tile([P, 4, 2, HID], dt_f1, name="w1_sb", tag="w1_sb")
        if FP8_F2:
            w2_sb = wpool.tile([P, 16, 2, D], dt_f2, name="w2_sb", tag="w2_sb")

        pAB = sAB.enter_context(tc.tile_pool(name="pAB", bufs=1))
        z1T = pAB.tile([P, 2, NBD, TL], dt_qkv, tag="z1T")

        # ---------------- Stage A: LN1 + transpose to z1T ----------------
        def ln_body(stpool, zpool, tpsum, xt, dstT, tcols, drain_idx,
                    split_stats=False):
            st = stpool.tile([P, 2, 6], F32, tag="ln_st")
            nc.vector.bn_stats(st[:, 0, :], xt[:, 0:512])
            nc.vector.bn_stats(st[:, 1, :], xt[:, 512:1024])
            mv = stpool.tile([P, 2], F32, tag="ln_mv")
            nc.vector.bn_aggr(mv, st)
            rstd = stpool.tile([P, 1], F32, tag="ln_rstd")
            nc.scalar.activation(rstd, mv[:, 1:2], AF.Sqrt, bias=eps_t, scale=1.0)
            nc.vector.reciprocal(rstd, rstd)
            zt = zpool.tile([P, D], F16, tag="ln_z")
            nc.vector.tensor_scalar(
                zt, xt, scalar1=mv[:, 0:1], scalar2=rstd,
                op0=ALU.subtract, op1=ALU.mult,
            )
            tp = tpsum.tile([P, NBD, P], F16, tag="ln_tp")
            for do in range(NBD):
                nc.tensor.transpose(tp[:, do, :], zt[:, ts(do, P)], idt)
            # one consolidated drain: [P, 8, 128] -> dstT[:, :, tcols]
            if drain_idx % 2 == 0:
                nc.vector.tensor_copy(dstT[:, :, tcols], tp)
            else:
                nc.scalar.activation(dstT[:, :, tcols], tp, AF.Copy,
                                     bias=0.0, scale=1.0)

        with ExitStack() as sA:
            xpool = sA.enter_context(tc.tile_pool(name="Ax", bufs=10))
            stpool = sA.enter_context(tc.tile_pool(name="Ast", bufs=4))
            zpool = sA.enter_context(tc.tile_pool(name="Az", bufs=2))
            tpsum = sA.enter_context(tc.tile_pool(name="Atp", bufs=3, space="PSUM"))
            xts = {}
            for s in range(2):   # prefetch all x tiles ahead of the weights
                for tt in range(NT):
                    xt = xpool.tile([P, D], F32, tag="ln_x")
                    nc.sync.dma_start(xt, xs[s, ts(tt, P), :])
                    xts[(s, tt)] = xt
            nc.sync.dma_start(wq_sb, wq)
            nc.sync.dma_start(wk_sb, wk)
            nc.sync.dma_start(wv_sb, wv)
            nc.sync.dma_start(wo_sb, wo)
            if FP8_F1:
                nc.sync.dma_start(w1_sb, w1)
            if FP8_F2:
                nc.sync.dma_start(w2_sb, w2)
            for s in range(2):
                for tt in range(NT):
                    ln_body(stpool, zpool, tpsum, xts[(s, tt)],
                            z1T[:, s], ts(tt, P), s * NT + tt)

        # ---------------- Stage B: QKV projections ----------------
        pBC = sBC.enter_context(tc.tile_pool(name="pBC", bufs=1, side="right"))
        qT = pBC.tile([P, 2, NBD, TOWN], F16, tag="qT")
        kT = pBC.tile([P, 2, NBD, TL], F16, tag="kT")
        # packed V per head pair: par0 = [V_e(0:64) ones(64) 0(65:96)],
        #                         par1 = [ones(0) 0(1:32) V_o(32:96)]
        vv2 = pBC.tile([P, 2, NT, NBD, 2, 96], F16, tag="vv2")
        nc.vector.memset(vv2[:, :, :, :, 0, 64:96], 0.0)
        nc.vector.memset(vv2[:, :, :, :, 1, 0:32], 0.0)
        nc.vector.memset(vv2[:, :, :, :, 0, 64:65], 1.0)
        nc.vector.memset(vv2[:, :, :, :, 1, 0:1], 1.0)

        z1v = z1T.rearrange("p s (a b) t -> p s a b t", b=2)
        uq = IWS if FP8_QKV else None

        def drain(idx, dst, src, scale):
            """dst = src * scale (optional); PSUM reads only on vector/scalar."""
            if idx % 2 == 1:
                nc.scalar.activation(dst, src, AF.Copy, bias=0.0,
                                     scale=scale if scale is not None else 1.0)
            elif scale is None:
                nc.vector.tensor_copy(dst, src)
            else:
                nc.vector.tensor_scalar(
                    dst, src, scalar1=scale, scalar2=None, op0=ALU.mult)

        with ExitStack() as sB:
            pp = sB.enter_context(tc.tile_pool(name="Bp", bufs=3, space="PSUM"))
            di = 0
            # Q^T d-major [qfeat, t_own]: two nb blocks per 2-bank psum
            for s in range(2):
                for nbg in range(NBD // 2):
                    ps = pp.tile([P, 1024], F32, tag="pp")
                    for nbl in range(2):
                        nb = nbg * 2 + nbl
                        mm_win(ps[:, ds(nbl * 512, 512)],
                               lambda o2: wq_sb[:, o2, :, ts(nb, P)],
                               lambda o2: z1v[:, s, o2, :, 128:TL],
                               4, FP8_QKV, skip=True)
                    if hbq:
                        for nbl in range(2):
                            nc.any.tensor_scalar(
                                qT[:, s, nbg * 2 + nbl, :],
                                ps[:, ds(nbl * 512, 512)], scalar1=uq or 1.0,
                                scalar2=bias_sb["bq"][:, nbg * 2 + nbl:
                                                      nbg * 2 + nbl + 1],
                                op0=ALU.mult, op1=ALU.add)
                    else:
                        drain(di, qT[:, s, ds(nbg * 2, 2), :],
                              ps.rearrange("p (a t) -> p a t", a=2), uq)
                        di += 1
            # K^T d-major [kfeat, t_all 640]: bank-aligned chunks 512 + 128
            for s in range(2):
                for nb in range(NBD):
                    ps = pp.tile([P, 1024], F32, tag="pp")
                    for (co, cw) in ((0, 512), (512, 128)):
                        mm_win(ps[:, ds(co, cw)],
                               lambda o2: wk_sb[:, o2, :, ts(nb, P)],
                               lambda o2: z1v[:, s, o2, :, ds(co, cw)],
                               4, FP8_QKV, skip=True)
                    if hbk:
                        nc.any.tensor_scalar(
                            kT[:, s, nb, :], ps[:, 0:640], scalar1=uq or 1.0,
                            scalar2=bias_sb["bk"][:, nb:nb + 1],
                            op0=ALU.mult, op1=ALU.add)
                    else:
                        drain(di, kT[:, s, nb, :], ps[:, 0:640], uq)
                        di += 1
            # V token-major, packed into vv2 with parity split
            for s in range(2):
                for tt in range(NT):
                    ps = pp.tile([P, 1024], F32, tag="pp")
                    for vc in range(4):
                        mm_win(ps[:, ds(vc * 256, 256)],
                               lambda o2: z1v[:, s, o2, :, ts(tt, P)],
                               lambda o2: wv_sb[:, o2, :, ds(vc * 256, 256)],
                               4, FP8_QKV, skip=True, first=(vc % 2 == 0))
                    src = ps.rearrange("p (h c d) -> p h c d", c=2, d=64)
                    for par in range(2):
                        dst = vv2[:, s, tt, :, par, ds(32 * par, 64)]
                        if hbv:
                            bsrc = bias_sb["bv"].rearrange(
                                "p (h c d) -> p h c d", c=2, d=64)
                            nc.any.tensor_scalar(
                                dst, src[:, :, par], scalar1=uq or 1.0,
                                scalar2=None, op0=ALU.mult)
                            nc.any.tensor_tensor(dst, dst, bsrc[:, :, par],
                                                 ALU.add)
                        else:
                            drain(di, dst, src[:, :, par], uq)
                            di += 1
        sAB.close()  # z1T dead

        # ---------------- Stage C: attention ----------------
        pCD = sCD.enter_context(tc.tile_pool(name="pCD", bufs=1))
        ctxT = pCD.tile([P, 2, NBD, TOWN], dt_o, tag="ctxT")
        with ExitStack() as sC:
            epool = sC.enter_context(tc.tile_pool(name="Ce", bufs=2))
            rpool = sC.enter_context(tc.tile_pool(name="Cr", bufs=2))
            r16p = sC.enter_context(tc.tile_pool(name="Cr16", bufs=2))
            sps = sC.enter_context(tc.tile_pool(name="Cs", bufs=2, space="PSUM"))
            cps = sC.enter_context(tc.tile_pool(name="Cc", bufs=2, space="PSUM"))
            reps = sC.enter_context(tc.tile_pool(name="Crp", bufs=2, space="PSUM"))
            for s in range(2):
                for h in range(H):
                    base = 64 * (h % 2)
                    hp = h // 2
                    odd = h % 2 == 1
                    # S^T strip: windowed matmuls into one 2-bank psum (key
                    # block 2 split at the bank boundary; one start per bank)
                    sp = sps.tile([P, 1024], F32, tag="sps")
                    for (off, w, kb, q0, st) in SWIN:
                        nc.tensor.matmul(
                            sp[:, ds(off, w)],
                            lhsT=kT[base:base + 64, s, hp, ts(kb, P)],
                            rhs=qT[base:base + 64, s, hp, ds(q0, w)],
                            start=st, stop=st, skip_group_check=True)
                    ET = epool.tile([P, 1024], F16, tag="ET")
                    nc.scalar.activation(ET, sp, AF.Exp, bias=0.0, scale=SCALE)
                    eng = nc.vector if h % 2 == 0 else nc.gpsimd
                    eng.tensor_tensor(ET, ET, mC, ALU.mult)
                    # ctx + sums: windowed accumulation, merged per key block
                    pc = cps.tile([P, 512], F32, tag="cps")
                    vvhp = vv2[:, s, :, hp].rearrange("p t a b -> p t (a b)")
                    lsl = ds(64, 128) if odd else ds(0, 65)
                    nr = 128 if odd else 65
                    for j in range(NT):
                        w0 = max(0, (j - 1) * 128)
                        nc.tensor.matmul(
                            pc[0:nr, ds(w0, JW[j])], lhsT=vvhp[:, j, lsl],
                            rhs=ET[:, ds(JOFF[j], JW[j])],
                            start=(j == 0), stop=(j == NT - 1),
                            skip_group_check=True)
                    # normalize: replicate sums row, approx-recip at base 0,
                    # mixed-base psum*sbuf multiply into ctxT
                    srow = 32 if odd else 64
                    s16 = r16p.tile([P, 512], F16, tag="s16")
                    nc.scalar.activation(s16[srow:srow + 1, :],
                                         pc[srow:srow + 1, :], AF.Copy,
                                         bias=0.0, scale=1.0)
                    prep = reps.tile([P, 512], F32, tag="prep")
                    nc.tensor.matmul(
                        prep[0:64, :], lhsT=ones16[srow:srow + 1, :],
                        rhs=s16[srow:srow + 1, :], start=True, stop=True)
                    rsb = rpool.tile([P, 512], F32, tag="rsb")
                    nc.vector.reciprocal_approx_fast(rsb[0:64, :],
                                                     prep[0:64, :])
                    crows = ds(64, 64) if odd else ds(0, 64)
                    nc.vector.tensor_tensor(
                        ctxT[base:base + 64, s, hp, :], pc[crows, :],
                        rsb[0:64, :], ALU.mult)
        sBC.close()  # qT/kT/vv2 dead

        # ---------------- Stage D: out-proj + residual -> o1 (SBUF) -------
        pDF = sDF.enter_context(tc.tile_pool(name="pDF", bufs=1, side="right"))
        o1 = pDF.tile([P, 2, NTO, D], F32, tag="o1")
        ctv = ctxT.rearrange("p s (a b) t -> p s a b t", b=2)
        uo = IWS if FP8_O else None
        with ExitStack() as sD:
            xrp = sD.enter_context(tc.tile_pool(name="Dx", bufs=3))
            pp2 = sD.enter_context(tc.tile_pool(name="Dp", bufs=3, space="PSUM"))
            for s in range(2):
                for tt in range(NTO):
                    ps = pp2.tile([P, 1024], F32, tag="pp2")
                    for mc in range(2):
                        mm_win(ps[:, ds(mc * 512, 512)],
                               lambda o2: ctv[:, s, o2, :, ts(tt, P)],
                               lambda o2: wo_sb[:, o2, :, ds(mc * 512, 512)],
                               4, FP8_O, skip=True)
                    xr = xrp.tile([P, D], F32, tag="xr")
                    nc.sync.dma_start(xr, xs[s, ds(128 + tt * P, P), :])
                    dst = o1[:, s, tt, :]
                    if uo is not None:
                        if tt % 2 == 0:
                            nc.scalar.activation(dst, ps, AF.Copy, bias=0.0,
                                                 scale=uo)
                            nc.vector.tensor_tensor(dst, dst, xr, ALU.add)
                        else:
                            nc.vector.tensor_scalar(dst, ps, scalar1=uo,
                                                    scalar2=None, op0=ALU.mult)
                            nc.gpsimd.tensor_tensor(dst, dst, xr, ALU.add)
                    else:
                        nc.vector.tensor_tensor(dst, ps, xr, ALU.add)
                    if hbo:
                        nc.any.tensor_tensor(dst, dst, bias_sb["bo"], ALU.add)
        sCD.close()  # ctxT dead

        # ---------------- Stage E: LN2 -> z2T ----------------
        z2T = pDF.tile([P, 2, NBD, TOWN], dt_f1, tag="z2T")
        with ExitStack() as sE:
            stp2 = sE.enter_context(tc.tile_pool(name="Est", bufs=4))
            zp2 = sE.enter_context(tc.tile_pool(name="Ez", bufs=2))
            tps2 = sE.enter_context(tc.tile_pool(name="Etp", bufs=3, space="PSUM"))
            for s in range(2):
                for tt in range(NTO):
                    ln_body(stp2, zp2, tps2, o1[:, s, tt, :], z2T[:, s],
                            ts(tt, P), s * NTO + tt, split_stats=True)

        # ---------------- Stage F: FFN ----------------
        h1T = pDF.tile([P, 2, NBH, TOWN], dt_f2, tag="h1T")
        z2v = z2T.rearrange("p s (a b) t -> p s a b t", b=2)
        u1 = IWS if FP8_F1 else 1.0
        with ExitStack() as sF:
            fp = sF.enter_context(tc.tile_pool(name="Fp", bufs=3, space="PSUM"))
            fw = sF.enter_context(tc.tile_pool(name="Fw", bufs=2))
            # fc1 (+gelu): two nb blocks per 2-bank psum, one gelu drain
            if FP8_F1:
                for s in range(2):
                    for nbg in range(NBH // 2):
                        ps = fp.tile([P, 1024], F32, tag="f_pp")
                        for nbl in range(2):
                            nb = nbg * 2 + nbl
                            mm_win(ps[:, ds(nbl * 512, 512)],
                                   lambda o2: w1_sb[:, o2, :, ts(nb, P)],
                                   lambda o2: z2v[:, s, o2, :, :],
                                   4, True, skip=True)
                        if hb1:
                            for nbl in range(2):
                                nb = nbg * 2 + nbl
                                nc.scalar.activation(
                                    h1T[:, s, nb, :], ps[:, ds(nbl * 512, 512)],
                                    AF.Gelu, bias=bias_sb["b1"][:, nb:nb + 1],
                                    scale=u1)
                        else:
                            nc.scalar.activation(
                                h1T[:, s, ds(nbg * 2, 2), :].rearrange(
                                    "p a t -> p (a t)"),
                                ps, AF.Gelu, bias=0.0, scale=u1)
            else:
                # fp16 fc1: stream w1 in four 1024-col slabs
                for nch in range(4):
                    wsb = fw.tile([P, 4, 2, 1024], F16, tag="w1slab")
                    nc.sync.dma_start(wsb, w1[:, :, :, ds(nch * 1024, 1024)])
                    for s in range(2):
                        for nbgl in range(4):
                            ps = fp.tile([P, 1024], F32, tag="f_pp")
                            for nbl in range(2):
                                nbw = nbgl * 2 + nbl
                                mm_win(ps[:, ds(nbl * 512, 512)],
                                       lambda o2: wsb[:, o2, :, ts(nbw, P)],
                                       lambda o2: z2v[:, s, o2, :, :],
                                       4, False, skip=True)
                            nb = nch * 8 + nbgl * 2
                            if hb1:
                                for nbl in range(2):
                                    nc.scalar.activation(
                                        h1T[:, s, nb + nbl, :],
                                        ps[:, ds(nbl * 512, 512)], AF.Gelu,
                                        bias=bias_sb["b1"][:, nb + nbl:
                                                           nb + nbl + 1],
                                        scale=u1)
                            else:
                                nc.scalar.activation(
                                    h1T[:, s, ds(nb, 2), :].rearrange(
                                        "p a t -> p (a t)"),
                                    ps, AF.Gelu, bias=0.0, scale=u1)

            # fc2 + residual -> ys
            h1v = h1T.rearrange("p s (a b) t -> p s a b t", b=2)
            u2 = IWS if FP8_F2 else None
            fyp = sF.enter_context(tc.tile_pool(name="Fy", bufs=3))
            if FP8_F2:
                for s in range(2):
                    for tt in range(NTO):
                        ps = fp.tile([P, 1024], F32, tag="f_pp")
                        for mc in range(2):
                            mm_win(ps[:, ds(mc * 512, 512)],
                                   lambda o2: h1v[:, s, o2, :, ts(tt, P)],
                                   lambda o2: w2_sb[:, o2, :, ds(mc * 512, 512)],
                                   16, True, skip=True)
                        yt = fyp.tile([P, D], F32, tag="yt")
                        if tt % 2 == 0:
                            nc.vector.tensor_scalar(yt, ps, scalar1=u2,
                                                    scalar2=None, op0=ALU.mult)
                        else:
                            nc.scalar.activation(yt, ps, AF.Copy, bias=0.0,
                                                 scale=u2)
                        nc.gpsimd.tensor_tensor(yt, yt, o1[:, s, tt, :],
                                                ALU.add)
                        if hb2:
                            nc.any.tensor_tensor(yt, yt, bias_sb["b2"], ALU.add)
                        nc.sync.dma_start(ys[s, ts(tt, P), :], yt)
            else:
                # fp16 fc2: stream w2 in two 512-col slabs
                for mc in range(2):
                    wsb2 = fw.tile([P, 16, 2, 512], F16, tag="w2slab")
                    nc.sync.dma_start(wsb2, w2[:, :, :, ds(mc * 512, 512)])
                    for s in range(2):
                        for tt in range(NTO):
                            ps = fp.tile([P, 1024], F32, tag="f_pp")
                            mm_win(ps[:, 0:512],
                                   lambda o2: h1v[:, s, o2, :, ts(tt, P)],
                                   lambda o2: wsb2[:, o2, :, :],
                                   16, False, skip=True)
                            yt = fyp.tile([P, 512], F32, tag="yt2")
                            o1r = o1[:, s, tt, ds(mc * 512, 512)]
                            nc.vector.tensor_copy(yt, ps[:, 0:512])
                            nc.gpsimd.tensor_tensor(yt, yt, o1r, ALU.add)
                            if hb2:
                                nc.any.tensor_tensor(
                                    yt, yt, bias_sb["b2"][:, ds(mc * 512, 512)],
                                    ALU.add)
                            nc.sync.dma_start(
                                ys[s, ts(tt, P), ds(mc * 512, 512)], yt)
        sDF.close()
        sW.close()
        sConst.close()

    nc.compile()
    return nc


_PROGRAM_CACHE = {}


def get_program(has_bias):
    key = tuple(has_bias)
    if key not in _PROGRAM_CACHE:
        nc = bacc.Bacc("TRN2", target_bir_lowering=False, debug=False,
                       num_devices=NCORES)
        _PROGRAM_CACHE[key] = _emit(nc, tuple(has_bias))
    return _PROGRAM_CACHE[key]


def _wlayout(w, ko, fp8):
    """[Kdim, N] -> [P, ko, 2, N] (contraction k = o2*256 + kt*128 + p)."""
    kdim, n = w.shape
    assert kdim == ko * 256
    wr = w.reshape(ko, 2, P, n).transpose(2, 0, 1, 3)
    wr = np.ascontiguousarray(wr)
    if fp8:
        return (wr * WS).astype(ml_dtypes.float8_e4m3)
    return wr.astype(np.float16)


def make_host_inputs(x, ln1_g, ln1_b, wq, wk, wv, bq, bk, bv, wo, bo,
                     ln2_g, ln2_b, w1, b1, w2, b2):
    """Fold LN affine params into weights, build per-core sharded inputs.
    Returns (in_maps, has_bias)."""
    f = np.float32
    x = np.asarray(x, f)
    wq_f = np.asarray(wq, f) * np.asarray(ln1_g, f)[:, None]
    wk_f = np.asarray(wk, f) * np.asarray(ln1_g, f)[:, None]
    wv_f = np.asarray(wv, f) * np.asarray(ln1_g, f)[:, None]
    w1_f = np.asarray(w1, f) * np.asarray(ln2_g, f)[:, None]
    bq_f = (np.asarray(bq, f) + np.asarray(ln1_b, f) @ np.asarray(wq, f)).astype(f)
    bk_f = (np.asarray(bk, f) + np.asarray(ln1_b, f) @ np.asarray(wk, f)).astype(f)
    bv_f = (np.asarray(bv, f) + np.asarray(ln1_b, f) @ np.asarray(wv, f)).astype(f)
    b1_f = (np.asarray(b1, f) + np.asarray(ln2_b, f) @ np.asarray(w1, f)).astype(f)
    bo_f = np.asarray(bo, f)
    b2_f = np.asarray(b2, f)
    wo_f = np.asarray(wo, f)
    w2_f = np.asarray(w2, f)

    has_bias = tuple(bool(np.any(v)) for v in (bq_f, bk_f, bv_f, bo_f, b1_f, b2_f))
    hbq, hbk, hbv, hbo, hb1, hb2 = has_bias

    wq_h = _wlayout(wq_f, 4, FP8_QKV)
    wk_h = _wlayout(wk_f, 4, FP8_QKV)
    wv_h = _wlayout(wv_f, 4, FP8_QKV)
    wo_h = _wlayout(wo_f, 4, FP8_O)
    w1_h = _wlayout(w1_f, 4, FP8_F1)
    w2_h = _wlayout(w2_f, 16, FP8_F2)

    r = np.arange(P)[:, None]    # k within block
    c = np.arange(256)[None, :]  # q within window
    band = ((c >= r) & (c <= r + 128)).astype(np.float16)
    m0 = band[:, 128:256]        # j=0: lower-tri (c <= r)
    m4 = band[:, 0:128]
    maskC_h = np.concatenate([m0, band, band, band, m4], axis=1)
    maskC_0 = maskC_h.copy()
    maskC_0[:, 0:128] = 0.0      # first chunk: halo block invalid
    ident = np.eye(P, dtype=np.float16)

    in_maps = []
    for core in range(NCORES):
        b, cchunk = divmod(core, NCORES // B)
        start = cchunk * CHUNK - HALO
        xsh = np.zeros((2, TL, D), f)
        for s in range(2):
            lo = start + s
            idx = lo + 2 * np.arange(TL)
            valid = idx >= 0
            xsh[s, valid] = x[b, idx[valid]]
        m = {
            "xs": xsh, "wq": wq_h, "wk": wk_h, "wv": wv_h, "wo": wo_h,
            "w1": w1_h, "w2": w2_h,
            "maskC": maskC_0 if cchunk == 0 else maskC_h,
            "ident": ident,
        }
        if hbq:
            m["bq"] = bq_f.reshape(NBD, P)
        if hbk:
            m["bk"] = bk_f.reshape(NBD, P)
        if hbv:
            m["bv"] = bv_f
        if hbo:
            m["bo"] = bo_f
        if hb1:
            m["b1"] = b1_f.reshape(NBH, P)
        if hb2:
            m["b2"] = b2_f
        in_maps.append(m)
    return in_maps, has_bias


def assemble_output(core_outs):
    """core_outs: list of 8 arrays [2, 512, D] -> full [B, L, D]."""
    out = np.empty((B, L, D), np.float32)
    for core, ysh in enumerate(core_outs):
        b, c = divmod(core, NCORES // B)
        for s in range(2):
            out[b, c * CHUNK + s: (c + 1) * CHUNK: 2, :] = ysh[s]
    return out


def run(inputs, trace=False):
    in_maps, has_bias = make_host_inputs(**inputs)
    nc = get_program(has_bias)
    from concourse.bass_utils import run_bass_kernel_spmd
    res = run_bass_kernel_spmd(nc, in_maps, core_ids=list(range(NCORES)),
                               trace=trace)
    out = assemble_output([r["ys"] for r in res.results])
    return out, res


def kernel(**inputs):
    out, _ = run(inputs, trace=False)
    return out


# revision 51
# speedup vs baseline: 1.1948x; 1.1948x over previous
# Trainium2 Bass kernel for nn_DilatedAttention (B=2, L=4096, D=1024, H=16,
# dilation=2, window=256): pre-LN attention block + FFN with residuals.
#
# Sharding: 8 cores = 2 batches x 4 sequence chunks of 1024 tokens, each with a
# 256-token halo on the left for K/V. No collectives. Dilated attention
# decomposes into two independent parity strands; within a strand it is a
# causal sliding-window attention with window 128 (+self).
#
# Key implementation choices:
#  - Projections run in fp8e4m3 DoubleRow (256-deep contraction per pass)
#    except groups toggled to fp16 for accuracy (weights then stream in
#    slabs). fp8 weights are scaled by 64 on host (into e4m3 normal range)
#    and unscaled at PSUM drain.
#  - fp8 weights are SBUF-resident for the whole kernel, one DMA each.
#  - Attention computes S^T[k,q] directly (lhsT=kT block, rhs=qT window) into
#    one 2-bank [128,1024] PSUM strip per (strand, head): a single exp and a
#    single concatenated-mask multiply cover all five key blocks. Probs are
#    k-major so the ctx matmul needs no transposes; softmax denominators come
#    from an appended ones-column in the packed V tile; ctx accumulates via
#    windowed matmuls with per-element has_written (mixed accumulate/first-
#    write inside one instruction), and is normalized by a replicated
#    fast-approx reciprocal on the way to ctxT.
#  - o1 (x + attn_out) stays in SBUF f32: no DRAM round trip.
#  - Scalar-engine table phases stay contiguous (sqrt / exp / sqrt / gelu).
import sys

sys.path.insert(0, "/opt/trn_rl_repo")

import os
from contextlib import ExitStack

import numpy as np
import ml_dtypes

import concourse.bass as bass
import concourse.mybir as mybir
import concourse.tile as tile
from concourse import bacc
from concourse.bass import ds, ts

F32 = mybir.dt.float32
F16 = mybir.dt.float16
FP8 = mybir.dt.float8e4
AF = mybir.ActivationFunctionType
ALU = mybir.AluOpType
DR = mybir.MatmulPerfMode.DoubleRow

B, L, D, H, HD, HID = 2, 4096, 1024, 16, 64, 4096
P = 128
NCORES = 8
CHUNK = 1024          # own tokens per core
HALO = 256            # original-token halo
TL = 640              # strand length incl halo (128 + 512)
TOWN = 512            # own strand tokens per parity
NBD = D // P          # 8 d-blocks
NBH = HID // P        # 32 hidden blocks
NT = TL // P          # 5 strand token tiles
NTO = TOWN // P       # 4 own token tiles
EPS = 1e-5
SCALE = 1.0 / 8.0     # 1/sqrt(HD)
WS = 64.0             # fp8 weight scale (host multiplies, kernel divides)
IWS = 1.0 / WS

# S^T column offsets for the five key blocks (widths 128,256,256,256,128)
JOFF = [0, 128, 384, 640, 896]
JW = [128, 256, 256, 256, 128]
# S^T emission windows (col_off, width, kblock, q_off, bank_first): key block 2
# is split at the PSUM bank boundary so each bank has exactly one start=True
SWIN = [(0, 128, 0, 0, True), (128, 256, 1, 0, False),
        (384, 128, 2, 128, False), (512, 128, 2, 256, True),
        (640, 256, 3, 256, False), (896, 128, 4, 384, False)]

# fc1 runs in fp16 (streamed weight slabs): the all-fp8 configuration exceeds
# the 2e-2 accuracy gate (measured 2.34e-2); qkv+o+fc2 in fp8 with fc1 fp16
# measures 1.81e-2.
_cfg = os.environ.get("FP8CFG") or "1101"
FP8_QKV = _cfg[0] == "1"
FP8_O = _cfg[1] == "1"
FP8_F1 = _cfg[2] == "1"
FP8_F2 = _cfg[3] == "1"


def _emit(nc, has_bias):
    hbq, hbk, hbv, hbo, hb1, hb2 = has_bias
    dt_qkv = FP8 if FP8_QKV else F16
    dt_o = FP8 if FP8_O else F16
    dt_f1 = FP8 if FP8_F1 else F16
    dt_f2 = FP8 if FP8_F2 else F16

    xs = nc.dram_tensor("xs", [2, TL, D], F32, kind="ExternalInput").ap()
    wq = nc.dram_tensor("wq", [P, 4, 2, D], dt_qkv, kind="ExternalInput").ap()
    wk = nc.dram_tensor("wk", [P, 4, 2, D], dt_qkv, kind="ExternalInput").ap()
    wv = nc.dram_tensor("wv", [P, 4, 2, D], dt_qkv, kind="ExternalInput").ap()
    wo = nc.dram_tensor("wo", [P, 4, 2, D], dt_o, kind="ExternalInput").ap()
    w1 = nc.dram_tensor("w1", [P, 4, 2, HID], dt_f1, kind="ExternalInput").ap()
    w2 = nc.dram_tensor("w2", [P, 16, 2, D], dt_f2, kind="ExternalInput").ap()
    maskC = nc.dram_tensor("maskC", [P, 1024], F16, kind="ExternalInput").ap()
    ident = nc.dram_tensor("ident", [P, P], F16, kind="ExternalInput").ap()
    bias_in = {}
    if hbq:
        bias_in["bq"] = nc.dram_tensor("bq", [NBD, P], F32, kind="ExternalInput").ap()
    if hbk:
        bias_in["bk"] = nc.dram_tensor("bk", [NBD, P], F32, kind="ExternalInput").ap()
    if hbv:
        bias_in["bv"] = nc.dram_tensor("bv", [D], F32, kind="ExternalInput").ap()
    if hbo:
        bias_in["bo"] = nc.dram_tensor("bo", [D], F32, kind="ExternalInput").ap()
    if hb1:
        bias_in["b1"] = nc.dram_tensor("b1", [NBH, P], F32, kind="ExternalInput").ap()
    if hb2:
        bias_in["b2"] = nc.dram_tensor("b2", [D], F32, kind="ExternalInput").ap()
    ys = nc.dram_tensor("ys", [2, TOWN, D], F32, kind="ExternalOutput").ap()

    def bcast(ap1d, n):
        return bass.AP(tensor=ap1d.tensor, offset=ap1d.offset, ap=[[0, P], *ap1d.ap])

    def mm_win(ps_win, lhsT_of, rhs_of, ko, fp8, skip=False, first=True):
        """Accumulation group over ko 256-blocks (fp8 DR) or 2ko 128-blocks.
        `first`: this group is the first writer of its PSUM bank (start=True).
        Only ONE start=True is allowed per 2KB bank region — it clears the
        whole bank's has_written state (probe5)."""
        if fp8:
            for o2 in range(ko):
                nc.tensor.matmul(ps_win, lhsT=lhsT_of(o2), rhs=rhs_of(o2),
                                 start=(first and o2 == 0), stop=(o2 == ko - 1),
                                 perf_mode=DR, skip_group_check=skip)
        else:
            for o2 in range(ko):
                la, ra = lhsT_of(o2), rhs_of(o2)
                for kt in range(2):
                    nc.tensor.matmul(
                        ps_win, lhsT=la[:, kt], rhs=ra[:, kt],
                        start=(first and o2 == 0 and kt == 0),
                        stop=(o2 == ko - 1 and kt == 1), skip_group_check=skip)

    with tile.TileContext(nc) as tc:
        sConst = ExitStack()
        sW = ExitStack()
        sAB = ExitStack()    # z1T
        sBC = ExitStack()    # qT, kT, vv2
        sCD = ExitStack()    # ctxT
        sDF = ExitStack()    # o1

        cpool = sConst.enter_context(tc.tile_pool(name="const", bufs=1))
        mC = cpool.tile([P, 1024], F16, tag="mC")
        nc.sync.dma_start(mC, maskC)
        idt = cpool.tile([P, P], F16, tag="idt")
        nc.sync.dma_start(idt, ident)
        ones16 = cpool.tile([P, 64], F16, tag="ones16")
        nc.vector.memset(ones16, 1.0)
        eps_t = cpool.tile([P, 1], F32, tag="eps")
        nc.vector.memset(eps_t, EPS)
        bias_sb = {}
        if hbq:
            bias_sb["bq"] = cpool.tile([P, NBD], F32, tag="bq")
            nc.sync.dma_start(bias_sb["bq"], bias_in["bq"].rearrange("o p -> p o"))
        if hbk:
            bias_sb["bk"] = cpool.tile([P, NBD], F32, tag="bk")
            nc.sync.dma_start(bias_sb["bk"], bias_in["bk"].rearrange("o p -> p o"))
        if hbv:
            bias_sb["bv"] = cpool.tile([P, D], F32, tag="bv")
            nc.sync.dma_start(bias_sb["bv"], bcast(bias_in["bv"], D))
        if hbo:
            bias_sb["bo"] = cpool.tile([P, D], F32, tag="bo")
            nc.sync.dma_start(bias_sb["bo"], bcast(bias_in["bo"], D))
        if hb1:
            bias_sb["b1"] = cpool.tile([P, NBH], F32, tag="b1")
            nc.sync.dma_start(bias_sb["b1"], bias_in["b1"].rearrange("o p -> p o"))
        if hb2:
            bias_sb["b2"] = cpool.tile([P, D], F32, tag="b2")
            nc.sync.dma_start(bias_sb["b2"], bcast(bias_in["b2"], D))

        # ---- resident weights; DMAs issued after stage A's x loads ----
        wpool = sW.enter_context(tc.tile_pool(name="wts", bufs=1, side="right"))
        wq_sb = wpool.tile([P, 4, 2, D], dt_qkv, tag="wq_sb")
        wk_sb = wpool.tile([P, 4, 2, D], dt_qkv, tag="wk_sb")
        wv_sb = wpool.tile([P, 4, 2, D], dt_qkv, tag="wv_sb")
        wo_sb = wpool.tile([P, 4, 2, D], dt_o, tag="wo_sb")
        w1_sb = w2_sb = None
        if FP8_F1:
            w1_sb = wpool.tile([P, 4, 2, HID], dt_f1, name="w1_sb", tag="w1_sb")
        if FP8_F2:
            w2_sb = wpool.tile([P, 16, 2, D], dt_f2, name="w2_sb", tag="w2_sb")

        pAB = sAB.enter_context(tc.tile_pool(name="pAB", bufs=1))
        z1T = pAB.tile([P, 2, NBD, TL], dt_qkv, tag="z1T")

        # ---------------- Stage A: LN1 + transpose to z1T ----------------
        def ln_body(stpool, zpool, tpsum, xt, dstT, tcols, drain_idx):
            st = stpool.tile([P, 2, 6], F32, tag="ln_st")
            nc.vector.bn_stats(st[:, 0, :], xt[:, 0:512])
            nc.vector.bn_stats(st[:, 1, :], xt[:, 512:1024])
            mv = stpool.tile([P, 2], F32, tag="ln_mv")
            nc.vector.bn_aggr(mv, st)
            rstd = stpool.tile([P, 1], F32, tag="ln_rstd")
            nc.scalar.activation(rstd, mv[:, 1:2], AF.Sqrt, bias=eps_t, scale=1.0)
            nc.vector.reciprocal(rstd, rstd)
            zt = zpool.tile([P, D], F16, tag="ln_z")
            nc.vector.tensor_scalar(
                zt, xt, scalar1=mv[:, 0:1], scalar2=rstd,
                op0=ALU.subtract, op1=ALU.mult,
            )
            tp = tpsum.tile([P, NBD, P], F16, tag="ln_tp")
            for do in range(NBD):
                nc.tensor.transpose(tp[:, do, :], zt[:, ts(do, P)], idt)
            # one consolidated drain: [P, 8, 128] -> dstT[:, :, tcols]
            if drain_idx % 2 == 0:
                nc.vector.tensor_copy(dstT[:, :, tcols], tp)
            else:
                nc.scalar.activation(dstT[:, :, tcols], tp, AF.Copy,
                                     bias=0.0, scale=1.0)

        with ExitStack() as sA:
            xpool = sA.enter_context(tc.tile_pool(name="Ax", bufs=10))
            stpool = sA.enter_context(tc.tile_pool(name="Ast", bufs=4))
            zpool = sA.enter_context(tc.tile_pool(name="Az", bufs=2))
            tpsum = sA.enter_context(tc.tile_pool(name="Atp", bufs=3, space="PSUM"))
            xts = {}
            for s in range(2):   # prefetch all x tiles ahead of the weights
                for tt in range(NT):
                    xt = xpool.tile([P, D], F32, tag="ln_x")
                    nc.sync.dma_start(xt, xs[s, ts(tt, P), :])
                    xts[(s, tt)] = xt
            nc.sync.dma_start(wq_sb, wq)
            nc.sync.dma_start(wk_sb, wk)
            nc.sync.dma_start(wv_sb, wv)
            nc.sync.dma_start(wo_sb, wo)
            if FP8_F1:
                nc.sync.dma_start(w1_sb, w1)
            if FP8_F2:
                nc.sync.dma_start(w2_sb, w2)
            for s in range(2):
                for tt in range(NT):
                    ln_body(stpool, zpool, tpsum, xts[(s, tt)],
                            z1T[:, s], ts(tt, P), s * NT + tt)

        # ---------------- Stage B: QKV projections ----------------
        pBC = sBC.enter_context(tc.tile_pool(name="pBC", bufs=1, side="right"))
        qT = pBC.tile([P, 2, NBD, TOWN], F16, tag="qT")
        kT = pBC.tile([P, 2, NBD, TL], F16, tag="kT")
        # packed V per head pair: par0 = [V_e(0:64) ones(64) 0(65:96)],
        #                         par1 = [ones(0) 0(1:32) V_o(32:96)]
        vv2 = pBC.tile([P, 2, NT, NBD, 2, 96], F16, tag="vv2")
        nc.vector.memset(vv2[:, :, :, :, 0, 64:96], 0.0)
        nc.vector.memset(vv2[:, :, :, :, 1, 0:32], 0.0)
        nc.vector.memset(vv2[:, :, :, :, 0, 64:65], 1.0)
        nc.vector.memset(vv2[:, :, :, :, 1, 0:1], 1.0)

        z1v = z1T.rearrange("p s (a b) t -> p s a b t", b=2)
        uq = IWS if FP8_QKV else None

        def drain(idx, dst, src, scale):
            """dst = src * scale (optional); PSUM reads only on vector/scalar."""
            if idx % 2 == 1:
                nc.scalar.activation(dst, src, AF.Copy, bias=0.0,
                                     scale=scale if scale is not None else 1.0)
            elif scale is None:
                nc.vector.tensor_copy(dst, src)
            else:
                nc.vector.tensor_scalar(
                    dst, src, scalar1=scale, scalar2=None, op0=ALU.mult)

        with ExitStack() as sB:
            pp = sB.enter_context(tc.tile_pool(name="Bp", bufs=3, space="PSUM"))
            di = 0
            # Q^T d-major [qfeat, t_own]: two nb blocks per 2-bank psum
            for s in range(2):
                for nbg in range(NBD // 2):
                    ps = pp.tile([P, 1024], F32, tag="pp")
                    for nbl in range(2):
                        nb = nbg * 2 + nbl
                        mm_win(ps[:, ds(nbl * 512, 512)],
                               lambda o2: wq_sb[:, o2, :, ts(nb, P)],
                               lambda o2: z1v[:, s, o2, :, 128:TL],
                               4, FP8_QKV, skip=True)
                    if hbq:
                        for nbl in range(2):
                            nc.any.tensor_scalar(
                                qT[:, s, nbg * 2 + nbl, :],
                                ps[:, ds(nbl * 512, 512)], scalar1=uq or 1.0,
                                scalar2=bias_sb["bq"][:, nbg * 2 + nbl:
                                                      nbg * 2 + nbl + 1],
                                op0=ALU.mult, op1=ALU.add)
                    else:
                        drain(di, qT[:, s, ds(nbg * 2, 2), :],
                              ps.rearrange("p (a t) -> p a t", a=2), uq)
                        di += 1
            # K^T d-major [kfeat, t_all 640]: bank-aligned chunks 512 + 128
            for s in range(2):
                for nb in range(NBD):
                    ps = pp.tile([P, 1024], F32, tag="pp")
                    for (co, cw) in ((0, 512), (512, 128)):
                        mm_win(ps[:, ds(co, cw)],
                               lambda o2: wk_sb[:, o2, :, ts(nb, P)],
                               lambda o2: z1v[:, s, o2, :, ds(co, cw)],
                               4, FP8_QKV, skip=True)
                    if hbk:
                        nc.any.tensor_scalar(
                            kT[:, s, nb, :], ps[:, 0:640], scalar1=uq or 1.0,
                            scalar2=bias_sb["bk"][:, nb:nb + 1],
                            op0=ALU.mult, op1=ALU.add)
                    else:
                        drain(di, kT[:, s, nb, :], ps[:, 0:640], uq)
                        di += 1
            # V token-major, packed into vv2 with parity split
            for s in range(2):
                for tt in range(NT):
                    ps = pp.tile([P, 1024], F32, tag="pp")
                    for vc in range(4):
                        mm_win(ps[:, ds(vc * 256, 256)],
                               lambda o2: z1v[:, s, o2, :, ts(tt, P)],
                               lambda o2: wv_sb[:, o2, :, ds(vc * 256, 256)],
                               4, FP8_QKV, skip=True, first=(vc % 2 == 0))
                    src = ps.rearrange("p (h c d) -> p h c d", c=2, d=64)
                    for par in range(2):
                        dst = vv2[:, s, tt, :, par, ds(32 * par, 64)]
                        if hbv:
                            bsrc = bias_sb["bv"].rearrange(
                                "p (h c d) -> p h c d", c=2, d=64)
                            nc.any.tensor_scalar(
                                dst, src[:, :, par], scalar1=uq or 1.0,
                                scalar2=None, op0=ALU.mult)
                            nc.any.tensor_tensor(dst, dst, bsrc[:, :, par],
                                                 ALU.add)
                        else:
                            drain(di, dst, src[:, :, par], uq)
                            di += 1
        sAB.close()  # z1T dead

        # ---------------- Stage C: attention ----------------
        pCD = sCD.enter_context(tc.tile_pool(name="pCD", bufs=1))
        ctxT = pCD.tile([P, 2, NBD, TOWN], dt_o, tag="ctxT")
        with ExitStack() as sC:
            epool = sC.enter_context(tc.tile_pool(name="Ce", bufs=2))
            rpool = sC.enter_context(tc.tile_pool(name="Cr", bufs=2))
            r16p = sC.enter_context(tc.tile_pool(name="Cr16", bufs=2))
            sps = sC.enter_context(tc.tile_pool(name="Cs", bufs=2, space="PSUM"))
            cps = sC.enter_context(tc.tile_pool(name="Cc", bufs=2, space="PSUM"))
            reps = sC.enter_context(tc.tile_pool(name="Crp", bufs=2, space="PSUM"))
            for s in range(2):
                for h in range(H):
                    base = 64 * (h % 2)
                    hp = h // 2
                    odd = h % 2 == 1
                    # S^T strip: windowed matmuls into one 2-bank psum (key
                    # block 2 split at the bank boundary; one start per bank)
                    sp = sps.tile([P, 1024], F32, tag="sps")
                    for (off, w, kb, q0, st) in SWIN:
                        nc.tensor.matmul(
                            sp[:, ds(off, w)],
                            lhsT=kT[base:base + 64, s, hp, ts(kb, P)],
                            rhs=qT[base:base + 64, s, hp, ds(q0, w)],
                            start=st, stop=st, skip_group_check=True)
                    ET = epool.tile([P, 1024], F16, tag="ET")
                    nc.scalar.activation(ET, sp, AF.Exp, bias=0.0, scale=SCALE)
                    eng = nc.vector if h % 2 == 0 else nc.gpsimd
                    eng.tensor_tensor(ET, ET, mC, ALU.mult)
                    # ctx + sums: windowed accumulation, merged per key block
                    pc = cps.tile([P, 512], F32, tag="cps")
                    vvhp = vv2[:, s, :, hp].rearrange("p t a b -> p t (a b)")
                    lsl = ds(64, 128) if odd else ds(0, 65)
                    nr = 128 if odd else 65
                    for j in range(NT):
                        w0 = max(0, (j - 1) * 128)
                        nc.tensor.matmul(
                            pc[0:nr, ds(w0, JW[j])], lhsT=vvhp[:, j, lsl],
                            rhs=ET[:, ds(JOFF[j], JW[j])],
                            start=(j == 0), stop=(j == NT - 1),
                            skip_group_check=True)
                    # normalize: replicate sums row, approx-recip at base 0,
                    # mixed-base psum*sbuf multiply into ctxT
                    srow = 32 if odd else 64
                    s16 = r16p.tile([P, 512], F16, tag="s16")
                    nc.scalar.activation(s16[srow:srow + 1, :],
                                         pc[srow:srow + 1, :], AF.Copy,
                                         bias=0.0, scale=1.0)
                    prep = reps.tile([P, 512], F32, tag="prep")
                    nc.tensor.matmul(
                        prep[0:64, :], lhsT=ones16[srow:srow + 1, :],
                        rhs=s16[srow:srow + 1, :], start=True, stop=True)
                    rsb = rpool.tile([P, 512], F32, tag="rsb")
                    nc.vector.reciprocal_approx_fast(rsb[0:64, :],
                                                     prep[0:64, :])
                    crows = ds(64, 64) if odd else ds(0, 64)
                    nc.vector.tensor_tensor(
                        ctxT[base:base + 64, s, hp, :], pc[crows, :],
                        rsb[0:64, :], ALU.mult)
        sBC.close()  # qT/kT/vv2 dead

        # ---------------- Stage D: out-proj + residual -> o1 (SBUF) -------
        pDF = sDF.enter_context(tc.tile_pool(name="pDF", bufs=1, side="right"))
        o1 = pDF.tile([P, 2, NTO, D], F32, tag="o1")
        ctv = ctxT.rearrange("p s (a b) t -> p s a b t", b=2)
        uo = IWS if FP8_O else None
        with ExitStack() as sD:
            xrp = sD.enter_context(tc.tile_pool(name="Dx", bufs=3))
            pp2 = sD.enter_context(tc.tile_pool(name="Dp", bufs=3, space="PSUM"))
            for s in range(2):
                for tt in range(NTO):
                    ps = pp2.tile([P, 1024], F32, tag="pp2")
                    for mc in range(2):
                        mm_win(ps[:, ds(mc * 512, 512)],
                               lambda o2: ctv[:, s, o2, :, ts(tt, P)],
                               lambda o2: wo_sb[:, o2, :, ds(mc * 512, 512)],
                               4, FP8_O, skip=True)
                    xr = xrp.tile([P, D], F32, tag="xr")
                    nc.sync.dma_start(xr, xs[s, ds(128 + tt * P, P), :])
                    dst = o1[:, s, tt, :]
                    if uo is not None:
                        if tt % 2 == 0:
                            nc.vector.tensor_scalar(dst, ps, scalar1=uo,
                                                    scalar2=None, op0=ALU.mult)
                        else:
                            nc.scalar.activation(dst, ps, AF.Copy, bias=0.0,
                                                 scale=uo)
                        nc.gpsimd.tensor_tensor(dst, dst, xr, ALU.add)
                    else:
                        nc.vector.tensor_tensor(dst, ps, xr, ALU.add)
                    if hbo:
                        nc.any.tensor_tensor(dst, dst, bias_sb["bo"], ALU.add)
        sCD.close()  # ctxT dead

        # ---------------- Stage E: LN2 -> z2T ----------------
        z2T = pDF.tile([P, 2, NBD, TOWN], dt_f1, tag="z2T")
        with ExitStack() as sE:
            stp2 = sE.enter_context(tc.tile_pool(name="Est", bufs=4))
            zp2 = sE.enter_context(tc.tile_pool(name="Ez", bufs=2))
            tps2 = sE.enter_context(tc.tile_pool(name="Etp", bufs=3, space="PSUM"))
            for s in range(2):
                for tt in range(NTO):
                    ln_body(stp2, zp2, tps2, o1[:, s, tt, :], z2T[:, s],
                            ts(tt, P), s * NTO + tt)

        # ---------------- Stage F: FFN ----------------
        h1T = pDF.tile([P, 2, NBH, TOWN], dt_f2, tag="h1T")
        z2v = z2T.rearrange("p s (a b) t -> p s a b t", b=2)
        u1 = IWS if FP8_F1 else 1.0
        with ExitStack() as sF:
            fp = sF.enter_context(tc.tile_pool(name="Fp", bufs=3, space="PSUM"))
            fw = sF.enter_context(tc.tile_pool(name="Fw", bufs=2))
            # fc1 (+gelu): two nb blocks per 2-bank psum, one gelu drain
            if FP8_F1:
                for s in range(2):
                    for nbg in range(NBH // 2):
                        ps = fp.tile([P, 1024], F32, tag="f_pp")
                        for nbl in range(2):
                            nb = nbg * 2 + nbl
                            mm_win(ps[:, ds(nbl * 512, 512)],
                                   lambda o2: w1_sb[:, o2, :, ts(nb, P)],
                                   lambda o2: z2v[:, s, o2, :, :],
                                   4, True, skip=True)
                        if hb1:
                            for nbl in range(2):
                                nb = nbg * 2 + nbl
                                nc.scalar.activation(
                                    h1T[:, s, nb, :], ps[:, ds(nbl * 512, 512)],
                                    AF.Gelu, bias=bias_sb["b1"][:, nb:nb + 1],
                                    scale=u1)
                        else:
                            nc.scalar.activation(
                                h1T[:, s, ds(nbg * 2, 2), :].rearrange(
                                    "p a t -> p (a t)"),
                                ps, AF.Gelu, bias=0.0, scale=u1)
            else:
                # fp16 fc1: stream w1 in four 1024-col slabs
                for nch in range(4):
                    wsb = fw.tile([P, 4, 2, 1024], F16, tag="w1slab")
                    nc.sync.dma_start(wsb, w1[:, :, :, ds(nch * 1024, 1024)])
                    for s in range(2):
                        for nbgl in range(4):
                            ps = fp.tile([P, 1024], F32, tag="f_pp")
                            for nbl in range(2):
                                nbw = nbgl * 2 + nbl
                                mm_win(ps[:, ds(nbl * 512, 512)],
                                       lambda o2: wsb[:, o2, :, ts(nbw, P)],
                                       lambda o2: z2v[:, s, o2, :, :],
                                       4, False, skip=True)
                            nb = nch * 8 + nbgl * 2
                            if hb1:
                                for nbl in range(2):
                                    nc.scalar.activation(
                                        h1T[:, s, nb + nbl, :],
                                        ps[:, ds(nbl * 512, 512)], AF.Gelu,
                                        bias=bias_sb["b1"][:, nb + nbl:
                                                           nb + nbl + 1],
                                        scale=u1)
                            else:
                                nc.scalar.activation(
                                    h1T[:, s, ds(nb, 2), :].rearrange(
                                        "p a t -> p (a t)"),
                                    ps, AF.Gelu, bias=0.0, scale=u1)

            # fc2 + residual -> ys
            h1v = h1T.rearrange("p s (a b) t -> p s a b t", b=2)
            u2 = IWS if FP8_F2 else None
            fyp = sF.enter_context(tc.tile_pool(name="Fy", bufs=3))
            if FP8_F2:
                for s in range(2):
                    for tt in range(NTO):
                        ps = fp.tile([P, 1024], F32, tag="f_pp")
                        for mc in range(2):
                            mm_win(ps[:, ds(mc * 512, 512)],
                                   lambda o2: h1v[:, s, o2, :, ts(tt, P)],
                                   lambda o2: w2_sb[:, o2, :, ds(mc * 512, 512)],
                                   16, True, skip=True)
                        yt = fyp.tile([P, D], F32, tag="yt")
                        if tt % 2 == 0:
                            nc.vector.tensor_scalar(yt, ps, scalar1=u2,
                                                    scalar2=None, op0=ALU.mult)
                        else:
                            nc.scalar.activation(yt, ps, AF.Copy, bias=0.0,
                                                 scale=u2)
                        nc.gpsimd.tensor_tensor(yt, yt, o1[:, s, tt, :],
                                                ALU.add)
                        if hb2:
                            nc.any.tensor_tensor(yt, yt, bias_sb["b2"], ALU.add)
                        nc.sync.dma_start(ys[s, ts(tt, P), :], yt)
            else:
                # fp16 fc2: stream w2 in two 512-col slabs
                for mc in range(2):
                    wsb2 = fw.tile([P, 16, 2, 512], F16, tag="w2slab")
                    nc.sync.dma_start(wsb2, w2[:, :, :, ds(mc * 512, 512)])
                    for s in range(2):
                        for tt in range(NTO):
                            ps = fp.tile([P, 1024], F32, tag="f_pp")
                            mm_win(ps[:, 0:512],
                                   lambda o2: h1v[:, s, o2, :, ts(tt, P)],
                                   lambda o2: wsb2[:, o2, :, :],
                                   16, False, skip=True)
                            yt = fyp.tile([P, 512], F32, tag="yt2")
                            o1r = o1[:, s, tt, ds(mc * 512, 512)]
                            nc.vector.tensor_copy(yt, ps[:, 0:512])
                            nc.gpsimd.tensor_tensor(yt, yt, o1r, ALU.add)
                            if hb2:
                                nc.any.tensor_tensor(
                                    yt, yt, bias_sb["b2"][:, ds(mc * 512, 512)],
                                    ALU.add)
                            nc.sync.dma_start(
                                ys[s, ts(tt, P), ds(mc * 512, 512)], yt)
        sDF.close()
        sW.close()
        sConst.close()

    nc.compile()
    return nc


_PROGRAM_CACHE = {}


def get_program(has_bias):
    key = tuple(has_bias)
    if key not in _PROGRAM_CACHE:
        nc = bacc.Bacc("TRN2", target_bir_lowering=False, debug=False,
                       num_devices=NCORES)
        _PROGRAM_CACHE[key] = _emit(nc, tuple(has_bias))
    return _PROGRAM_CACHE[key]


def _wlayout(w, ko, fp8):
    """[Kdim, N] -> [P, ko, 2, N] (contraction k = o2*256 + kt*128 + p)."""
    kdim, n = w.shape
    assert kdim == ko * 256
    wr = w.reshape(ko, 2, P, n).transpose(2, 0, 1, 3)
    wr = np.ascontiguousarray(wr)
    if fp8:
        return (wr * WS).astype(ml_dtypes.float8_e4m3)
    return wr.astype(np.float16)


def make_host_inputs(x, ln1_g, ln1_b, wq, wk, wv, bq, bk, bv, wo, bo,
                     ln2_g, ln2_b, w1, b1, w2, b2):
    """Fold LN affine params into weights, build per-core sharded inputs.
    Returns (in_maps, has_bias)."""
    f = np.float32
    x = np.asarray(x, f)
    wq_f = np.asarray(wq, f) * np.asarray(ln1_g, f)[:, None]
    wk_f = np.asarray(wk, f) * np.asarray(ln1_g, f)[:, None]
    wv_f = np.asarray(wv, f) * np.asarray(ln1_g, f)[:, None]
    w1_f = np.asarray(w1, f) * np.asarray(ln2_g, f)[:, None]
    bq_f = (np.asarray(bq, f) + np.asarray(ln1_b, f) @ np.asarray(wq, f)).astype(f)
    bk_f = (np.asarray(bk, f) + np.asarray(ln1_b, f) @ np.asarray(wk, f)).astype(f)
    bv_f = (np.asarray(bv, f) + np.asarray(ln1_b, f) @ np.asarray(wv, f)).astype(f)
    b1_f = (np.asarray(b1, f) + np.asarray(ln2_b, f) @ np.asarray(w1, f)).astype(f)
    bo_f = np.asarray(bo, f)
    b2_f = np.asarray(b2, f)
    wo_f = np.asarray(wo, f)
    w2_f = np.asarray(w2, f)

    has_bias = tuple(bool(np.any(v)) for v in (bq_f, bk_f, bv_f, bo_f, b1_f, b2_f))
    hbq, hbk, hbv, hbo, hb1, hb2 = has_bias

    wq_h = _wlayout(wq_f, 4, FP8_QKV)
    wk_h = _wlayout(wk_f, 4, FP8_QKV)
    wv_h = _wlayout(wv_f, 4, FP8_QKV)
    wo_h = _wlayout(wo_f, 4, FP8_O)
    w1_h = _wlayout(w1_f, 4, FP8_F1)
    w2_h = _wlayout(w2_f, 16, FP8_F2)

    r = np.arange(P)[:, None]    # k within block
    c = np.arange(256)[None, :]  # q within window
    band = ((c >= r) & (c <= r + 128)).astype(np.float16)
    m0 = band[:, 128:256]        # j=0: lower-tri (c <= r)
    m4 = band[:, 0:128]
    maskC_h = np.concatenate([m0, band, band, band, m4], axis=1)
    maskC_0 = maskC_h.copy()
    maskC_0[:, 0:128] = 0.0      # first chunk: halo block invalid
    ident = np.eye(P, dtype=np.float16)

    in_maps = []
    for core in range(NCORES):
        b, cchunk = divmod(core, NCORES // B)
        start = cchunk * CHUNK - HALO
        xsh = np.zeros((2, TL, D), f)
        for s in range(2):
            lo = start + s
            idx = lo + 2 * np.arange(TL)
            valid = idx >= 0
            xsh[s, valid] = x[b, idx[valid]]
        m = {
            "xs": xsh, "wq": wq_h, "wk": wk_h, "wv": wv_h, "wo": wo_h,
            "w1": w1_h, "w2": w2_h,
            "maskC": maskC_0 if cchunk == 0 else maskC_h,
            "ident": ident,
        }
        if hbq:
            m["bq"] = bq_f.reshape(NBD, P)
        if hbk:
            m["bk"] = bk_f.reshape(NBD, P)
        if hbv:
            m["bv"] = bv_f
        if hbo:
            m["bo"] = bo_f
        if hb1:
            m["b1"] = b1_f.reshape(NBH, P)
        if hb2:
            m["b2"] = b2_f
        in_maps.append(m)
    return in_maps, has_bias


def assemble_output(core_outs):
    """core_outs: list of 8 arrays [2, 512, D] -> full [B, L, D]."""
    out = np.empty((B, L, D), np.float32)
    for core, ysh in enumerate(core_outs):
        b, c = divmod(core, NCORES // B)
        for s in range(2):
            out[b, c * CHUNK + s: (c + 1) * CHUNK: 2, :] = ysh[s]
    return out


def run(inputs, trace=False):
    in_maps, has_bias = make_host_inputs(**inputs)
    nc = get_program(has_bias)
    from concourse.bass_utils import run_bass_kernel_spmd
    res = run_bass_kernel_spmd(nc, in_maps, core_ids=list(range(NCORES)),
                               trace=trace)
    out = assemble_output([r["ys"] for r in res.results])
    return out, res


def kernel(**inputs):
    out, _ = run(inputs, trace=False)
    return out


# revision 52
# speedup vs baseline: 1.2005x; 1.0048x over previous
# Trainium2 Bass kernel for nn_DilatedAttention (B=2, L=4096, D=1024, H=16,
# dilation=2, window=256): pre-LN attention block + FFN with residuals.
#
# Sharding: 8 cores = 2 batches x 4 sequence chunks of 1024 tokens, each with a
# 256-token halo on the left for K/V. No collectives. Dilated attention
# decomposes into two independent parity strands; within a strand it is a
# causal sliding-window attention with window 128 (+self).
#
# Key implementation choices:
#  - Projections run in fp8e4m3 DoubleRow (256-deep contraction per pass)
#    except groups toggled to fp16 for accuracy (weights then stream in
#    slabs). fp8 weights are scaled by 64 on host (into e4m3 normal range)
#    and unscaled at PSUM drain.
#  - fp8 weights are SBUF-resident for the whole kernel, one DMA each.
#  - Attention computes S^T[k,q] directly (lhsT=kT block, rhs=qT window) into
#    one 2-bank [128,1024] PSUM strip per (strand, head): a single exp and a
#    single concatenated-mask multiply cover all five key blocks. Probs are
#    k-major so the ctx matmul needs no transposes; softmax denominators come
#    from an appended ones-column in the packed V tile; ctx accumulates via
#    windowed matmuls with per-element has_written (mixed accumulate/first-
#    write inside one instruction), and is normalized by a replicated
#    fast-approx reciprocal on the way to ctxT.
#  - o1 (x + attn_out) stays in SBUF f32: no DRAM round trip.
#  - Scalar-engine table phases stay contiguous (sqrt / exp / sqrt / gelu).
import sys

sys.path.insert(0, "/opt/trn_rl_repo")

import os
from contextlib import ExitStack

import numpy as np
import ml_dtypes

import concourse.bass as bass
import concourse.mybir as mybir
import concourse.tile as tile
from concourse import bacc
from concourse.bass import ds, ts

F32 = mybir.dt.float32
F16 = mybir.dt.float16
FP8 = mybir.dt.float8e4
AF = mybir.ActivationFunctionType
ALU = mybir.AluOpType
DR = mybir.MatmulPerfMode.DoubleRow

B, L, D, H, HD, HID = 2, 4096, 1024, 16, 64, 4096
P = 128
NCORES = 8
CHUNK = 1024          # own tokens per core
HALO = 256            # original-token halo
TL = 640              # strand length incl halo (128 + 512)
TOWN = 512            # own strand tokens per parity
NBD = D // P          # 8 d-blocks
NBH = HID // P        # 32 hidden blocks
NT = TL // P          # 5 strand token tiles
NTO = TOWN // P       # 4 own token tiles
EPS = 1e-5
SCALE = 1.0 / 8.0     # 1/sqrt(HD)
WS = 64.0             # fp8 weight scale (host multiplies, kernel divides)
IWS = 1.0 / WS

# S^T column offsets for the five key blocks (widths 128,256,256,256,128)
JOFF = [0, 128, 384, 640, 896]
JW = [128, 256, 256, 256, 128]
# S^T emission windows (col_off, width, kblock, q_off, bank_first): key block 2
# is split at the PSUM bank boundary so each bank has exactly one start=True
SWIN = [(0, 128, 0, 0, True), (128, 256, 1, 0, False),
        (384, 128, 2, 128, False), (512, 128, 2, 256, True),
        (640, 256, 3, 256, False), (896, 128, 4, 384, False)]

# fc1 runs in fp16 (streamed weight slabs): the all-fp8 configuration exceeds
# the 2e-2 accuracy gate (measured 2.34e-2); qkv+o+fc2 in fp8 with fc1 fp16
# measures 1.81e-2.
_cfg = os.environ.get("FP8CFG") or "1101"
FP8_QKV = _cfg[0] == "1"
FP8_O = _cfg[1] == "1"
FP8_F1 = _cfg[2] == "1"
FP8_F2 = _cfg[3] == "1"


def _emit(nc, has_bias):
    hbq, hbk, hbv, hbo, hb1, hb2 = has_bias
    dt_qkv = FP8 if FP8_QKV else F16
    dt_o = FP8 if FP8_O else F16
    dt_f1 = FP8 if FP8_F1 else F16
    dt_f2 = FP8 if FP8_F2 else F16

    xs = nc.dram_tensor("xs", [2, TL, D], F32, kind="ExternalInput").ap()
    wq = nc.dram_tensor("wq", [P, 4, 2, D], dt_qkv, kind="ExternalInput").ap()
    wk = nc.dram_tensor("wk", [P, 4, 2, D], dt_qkv, kind="ExternalInput").ap()
    wv = nc.dram_tensor("wv", [P, 4, 2, D], dt_qkv, kind="ExternalInput").ap()
    wo = nc.dram_tensor("wo", [P, 4, 2, D], dt_o, kind="ExternalInput").ap()
    w1 = nc.dram_tensor("w1", [P, 4, 2, HID], dt_f1, kind="ExternalInput").ap()
    w2 = nc.dram_tensor("w2", [P, 16, 2, D], dt_f2, kind="ExternalInput").ap()
    maskC = nc.dram_tensor("maskC", [P, 1024], F16, kind="ExternalInput").ap()
    ident = nc.dram_tensor("ident", [P, P], F16, kind="ExternalInput").ap()
    bias_in = {}
    if hbq:
        bias_in["bq"] = nc.dram_tensor("bq", [NBD, P], F32, kind="ExternalInput").ap()
    if hbk:
        bias_in["bk"] = nc.dram_tensor("bk", [NBD, P], F32, kind="ExternalInput").ap()
    if hbv:
        bias_in["bv"] = nc.dram_tensor("bv", [D], F32, kind="ExternalInput").ap()
    if hbo:
        bias_in["bo"] = nc.dram_tensor("bo", [D], F32, kind="ExternalInput").ap()
    if hb1:
        bias_in["b1"] = nc.dram_tensor("b1", [NBH, P], F32, kind="ExternalInput").ap()
    if hb2:
        bias_in["b2"] = nc.dram_tensor("b2", [D], F32, kind="ExternalInput").ap()
    ys = nc.dram_tensor("ys", [2, TOWN, D], F32, kind="ExternalOutput").ap()

    def bcast(ap1d, n):
        return bass.AP(tensor=ap1d.tensor, offset=ap1d.offset, ap=[[0, P], *ap1d.ap])

    def mm_win(ps_win, lhsT_of, rhs_of, ko, fp8, skip=False, first=True):
        """Accumulation group over ko 256-blocks (fp8 DR) or 2ko 128-blocks.
        `first`: this group is the first writer of its PSUM bank (start=True).
        Only ONE start=True is allowed per 2KB bank region — it clears the
        whole bank's has_written state (probe5)."""
        if fp8:
            for o2 in range(ko):
                nc.tensor.matmul(ps_win, lhsT=lhsT_of(o2), rhs=rhs_of(o2),
                                 start=(first and o2 == 0), stop=(o2 == ko - 1),
                                 perf_mode=DR, skip_group_check=skip)
        else:
            for o2 in range(ko):
                la, ra = lhsT_of(o2), rhs_of(o2)
                for kt in range(2):
                    nc.tensor.matmul(
                        ps_win, lhsT=la[:, kt], rhs=ra[:, kt],
                        start=(first and o2 == 0 and kt == 0),
                        stop=(o2 == ko - 1 and kt == 1), skip_group_check=skip)

    with tile.TileContext(nc) as tc:
        sConst = ExitStack()
        sW = ExitStack()
        sAB = ExitStack()    # z1T
        sBC = ExitStack()    # qT, kT, vv2
        sCD = ExitStack()    # ctxT
        sDF = ExitStack()    # o1

        cpool = sConst.enter_context(tc.tile_pool(name="const", bufs=1))
        mC = cpool.tile([P, 1024], F16, tag="mC")
        nc.sync.dma_start(mC, maskC)
        idt = cpool.tile([P, P], F16, tag="idt")
        nc.sync.dma_start(idt, ident)
        ones16 = cpool.tile([P, 64], F16, tag="ones16")
        nc.vector.memset(ones16, 1.0)
        eps_t = cpool.tile([P, 1], F32, tag="eps")
        nc.vector.memset(eps_t, EPS)
        bias_sb = {}
        if hbq:
            bias_sb["bq"] = cpool.tile([P, NBD], F32, tag="bq")
            nc.sync.dma_start(bias_sb["bq"], bias_in["bq"].rearrange("o p -> p o"))
        if hbk:
            bias_sb["bk"] = cpool.tile([P, NBD], F32, tag="bk")
            nc.sync.dma_start(bias_sb["bk"], bias_in["bk"].rearrange("o p -> p o"))
        if hbv:
            bias_sb["bv"] = cpool.tile([P, D], F32, tag="bv")
            nc.sync.dma_start(bias_sb["bv"], bcast(bias_in["bv"], D))
        if hbo:
            bias_sb["bo"] = cpool.tile([P, D], F32, tag="bo")
            nc.sync.dma_start(bias_sb["bo"], bcast(bias_in["bo"], D))
        if hb1:
            bias_sb["b1"] = cpool.tile([P, NBH], F32, tag="b1")
            nc.sync.dma_start(bias_sb["b1"], bias_in["b1"].rearrange("o p -> p o"))
        if hb2:
            bias_sb["b2"] = cpool.tile([P, D], F32, tag="b2")
            nc.sync.dma_start(bias_sb["b2"], bcast(bias_in["b2"], D))

        # ---- resident weights; DMAs issued after stage A's x loads ----
        wpool = sW.enter_context(tc.tile_pool(name="wts", bufs=1, side="right"))
        wq_sb = wpool.tile([P, 4, 2, D], dt_qkv, tag="wq_sb")
        wk_sb = wpool.tile([P, 4, 2, D], dt_qkv, tag="wk_sb")
        wv_sb = wpool.tile([P, 4, 2, D], dt_qkv, tag="wv_sb")
        wo_sb = wpool.tile([P, 4, 2, D], dt_o, tag="wo_sb")
        w1_sb = w2_sb = None
        if FP8_F1:
            w1_sb = wpool.tile([P, 4, 2, HID], dt_f1, name="w1_sb", tag="w1_sb")
        if FP8_F2:
            w2_sb = wpool.tile([P, 16, 2, D], dt_f2, name="w2_sb", tag="w2_sb")

        pAB = sAB.enter_context(tc.tile_pool(name="pAB", bufs=1))
        z1T = pAB.tile([P, 2, NBD, TL], dt_qkv, tag="z1T")

        # ---------------- Stage A: LN1 + transpose to z1T ----------------
        def ln_body(stpool, zpool, tpsum, xt, dstT, tcols, drain_idx):
            st = stpool.tile([P, 2, 6], F32, tag="ln_st")
            nc.vector.bn_stats(st[:, 0, :], xt[:, 0:512])
            nc.vector.bn_stats(st[:, 1, :], xt[:, 512:1024])
            mv = stpool.tile([P, 2], F32, tag="ln_mv")
            nc.vector.bn_aggr(mv, st)
            rstd = stpool.tile([P, 1], F32, tag="ln_rstd")
            nc.scalar.activation(rstd, mv[:, 1:2], AF.Sqrt, bias=eps_t, scale=1.0)
            nc.vector.reciprocal(rstd, rstd)
            zt = zpool.tile([P, D], F16, tag="ln_z")
            nc.vector.tensor_scalar(
                zt, xt, scalar1=mv[:, 0:1], scalar2=rstd,
                op0=ALU.subtract, op1=ALU.mult,
            )
            tp = tpsum.tile([P, NBD, P], F16, tag="ln_tp")
            for do in range(NBD):
                nc.tensor.transpose(tp[:, do, :], zt[:, ts(do, P)], idt)
            # one consolidated drain: [P, 8, 128] -> dstT[:, :, tcols]
            if drain_idx % 2 == 0:
                nc.vector.tensor_copy(dstT[:, :, tcols], tp)
            else:
                nc.scalar.activation(dstT[:, :, tcols], tp, AF.Copy,
                                     bias=0.0, scale=1.0)

        with ExitStack() as sA:
            xpool = sA.enter_context(tc.tile_pool(name="Ax", bufs=10))
            stpool = sA.enter_context(tc.tile_pool(name="Ast", bufs=4))
            zpool = sA.enter_context(tc.tile_pool(name="Az", bufs=2))
            tpsum = sA.enter_context(tc.tile_pool(name="Atp", bufs=3, space="PSUM"))
            xts = {}
            for s in range(2):   # prefetch all x tiles ahead of the weights
                for tt in range(NT):
                    xt = xpool.tile([P, D], F32, tag="ln_x")
                    nc.sync.dma_start(xt, xs[s, ts(tt, P), :])
                    xts[(s, tt)] = xt
            nc.sync.dma_start(wq_sb, wq)
            nc.sync.dma_start(wk_sb, wk)
            nc.sync.dma_start(wv_sb, wv)
            nc.sync.dma_start(wo_sb, wo)
            if FP8_F1:
                nc.sync.dma_start(w1_sb, w1)
            if FP8_F2:
                nc.sync.dma_start(w2_sb, w2)
            for s in range(2):
                for tt in range(NT):
                    ln_body(stpool, zpool, tpsum, xts[(s, tt)],
                            z1T[:, s], ts(tt, P), s * NT + tt)

        # ---------------- Stage B: QKV projections ----------------
        pBC = sBC.enter_context(tc.tile_pool(name="pBC", bufs=1, side="right"))
        qT = pBC.tile([P, 2, NBD, TOWN], F16, tag="qT")
        kT = pBC.tile([P, 2, NBD, TL], F16, tag="kT")
        # packed V per head pair: par0 = [V_e(0:64) ones(64) 0(65:96)],
        #                         par1 = [ones(0) 0(1:32) V_o(32:96)]
        vv2 = pBC.tile([P, 2, NT, NBD, 2, 96], F16, tag="vv2")
        nc.vector.memset(vv2[:, :, :, :, 0, 64:96], 0.0)
        nc.vector.memset(vv2[:, :, :, :, 1, 0:32], 0.0)
        nc.vector.memset(vv2[:, :, :, :, 0, 64:65], 1.0)
        nc.vector.memset(vv2[:, :, :, :, 1, 0:1], 1.0)

        z1v = z1T.rearrange("p s (a b) t -> p s a b t", b=2)
        uq = IWS if FP8_QKV else None

        def drain(idx, dst, src, scale):
            """dst = src * scale (optional); PSUM reads only on vector/scalar."""
            if idx % 2 == 1:
                nc.scalar.activation(dst, src, AF.Copy, bias=0.0,
                                     scale=scale if scale is not None else 1.0)
            elif scale is None:
                nc.vector.tensor_copy(dst, src)
            else:
                nc.vector.tensor_scalar(
                    dst, src, scalar1=scale, scalar2=None, op0=ALU.mult)

        with ExitStack() as sB:
            pp = sB.enter_context(tc.tile_pool(name="Bp", bufs=3, space="PSUM"))
            di = 0
            # Q^T d-major [qfeat, t_own]: two nb blocks per 2-bank psum
            for s in range(2):
                for nbg in range(NBD // 2):
                    ps = pp.tile([P, 1024], F32, tag="pp")
                    for nbl in range(2):
                        nb = nbg * 2 + nbl
                        mm_win(ps[:, ds(nbl * 512, 512)],
                               lambda o2: wq_sb[:, o2, :, ts(nb, P)],
                               lambda o2: z1v[:, s, o2, :, 128:TL],
                               4, FP8_QKV, skip=True)
                    if hbq:
                        for nbl in range(2):
                            nc.any.tensor_scalar(
                                qT[:, s, nbg * 2 + nbl, :],
                                ps[:, ds(nbl * 512, 512)], scalar1=uq or 1.0,
                                scalar2=bias_sb["bq"][:, nbg * 2 + nbl:
                                                      nbg * 2 + nbl + 1],
                                op0=ALU.mult, op1=ALU.add)
                    else:
                        drain(di, qT[:, s, ds(nbg * 2, 2), :],
                              ps.rearrange("p (a t) -> p a t", a=2), uq)
                        di += 1
            # K^T d-major [kfeat, t_all 640]: bank-aligned chunks 512 + 128
            for s in range(2):
                for nb in range(NBD):
                    ps = pp.tile([P, 1024], F32, tag="pp")
                    for (co, cw) in ((0, 512), (512, 128)):
                        mm_win(ps[:, ds(co, cw)],
                               lambda o2: wk_sb[:, o2, :, ts(nb, P)],
                               lambda o2: z1v[:, s, o2, :, ds(co, cw)],
                               4, FP8_QKV, skip=True)
                    if hbk:
                        nc.any.tensor_scalar(
                            kT[:, s, nb, :], ps[:, 0:640], scalar1=uq or 1.0,
                            scalar2=bias_sb["bk"][:, nb:nb + 1],
                            op0=ALU.mult, op1=ALU.add)
                    else:
                        drain(di, kT[:, s, nb, :], ps[:, 0:640], uq)
                        di += 1
            # V token-major, packed into vv2 with parity split
            for s in range(2):
                for tt in range(NT):
                    ps = pp.tile([P, 1024], F32, tag="pp")
                    for vc in range(4):
                        mm_win(ps[:, ds(vc * 256, 256)],
                               lambda o2: z1v[:, s, o2, :, ts(tt, P)],
                               lambda o2: wv_sb[:, o2, :, ds(vc * 256, 256)],
                               4, FP8_QKV, skip=True, first=(vc % 2 == 0))
                    src = ps.rearrange("p (h c d) -> p h c d", c=2, d=64)
                    for par in range(2):
                        dst = vv2[:, s, tt, :, par, ds(32 * par, 64)]
                        if hbv:
                            bsrc = bias_sb["bv"].rearrange(
                                "p (h c d) -> p h c d", c=2, d=64)
                            nc.any.tensor_scalar(
                                dst, src[:, :, par], scalar1=uq or 1.0,
                                scalar2=None, op0=ALU.mult)
                            nc.any.tensor_tensor(dst, dst, bsrc[:, :, par],
                                                 ALU.add)
                        else:
                            drain(di, dst, src[:, :, par], uq)
                            di += 1
        sAB.close()  # z1T dead

        # ---------------- Stage C: attention ----------------
        pCD = sCD.enter_context(tc.tile_pool(name="pCD", bufs=1))
        ctxT = pCD.tile([P, 2, NBD, TOWN], dt_o, tag="ctxT")
        with ExitStack() as sC:
            epool = sC.enter_context(tc.tile_pool(name="Ce", bufs=3))
            rpool = sC.enter_context(tc.tile_pool(name="Cr", bufs=2))
            r16p = sC.enter_context(tc.tile_pool(name="Cr16", bufs=2))
            sps = sC.enter_context(tc.tile_pool(name="Cs", bufs=2, space="PSUM"))
            cps = sC.enter_context(tc.tile_pool(name="Cc", bufs=2, space="PSUM"))
            reps = sC.enter_context(tc.tile_pool(name="Crp", bufs=2, space="PSUM"))
            for s in range(2):
                for h in range(H):
                    base = 64 * (h % 2)
                    hp = h // 2
                    odd = h % 2 == 1
                    # S^T strip: windowed matmuls into one 2-bank psum (key
                    # block 2 split at the bank boundary; one start per bank)
                    sp = sps.tile([P, 1024], F32, tag="sps")
                    for (off, w, kb, q0, st) in SWIN:
                        nc.tensor.matmul(
                            sp[:, ds(off, w)],
                            lhsT=kT[base:base + 64, s, hp, ts(kb, P)],
                            rhs=qT[base:base + 64, s, hp, ds(q0, w)],
                            start=st, stop=st, skip_group_check=True)
                    ET = epool.tile([P, 1024], F16, tag="ET")
                    nc.scalar.activation(ET, sp, AF.Exp, bias=0.0, scale=SCALE)
                    eng = nc.vector if h % 2 == 0 else nc.gpsimd
                    eng.tensor_tensor(ET, ET, mC, ALU.mult)
                    # ctx + sums: windowed accumulation, merged per key block
                    pc = cps.tile([P, 512], F32, tag="cps")
                    vvhp = vv2[:, s, :, hp].rearrange("p t a b -> p t (a b)")
                    lsl = ds(64, 128) if odd else ds(0, 65)
                    nr = 128 if odd else 65
                    for j in range(NT):
                        w0 = max(0, (j - 1) * 128)
                        nc.tensor.matmul(
                            pc[0:nr, ds(w0, JW[j])], lhsT=vvhp[:, j, lsl],
                            rhs=ET[:, ds(JOFF[j], JW[j])],
                            start=(j == 0), stop=(j == NT - 1),
                            skip_group_check=True)
                    # normalize: replicate sums row, approx-recip at base 0,
                    # mixed-base psum*sbuf multiply into ctxT
                    srow = 32 if odd else 64
                    s16 = r16p.tile([P, 512], F16, tag="s16")
                    nc.scalar.activation(s16[srow:srow + 1, :],
                                         pc[srow:srow + 1, :], AF.Copy,
                                         bias=0.0, scale=1.0)
                    prep = reps.tile([P, 512], F32, tag="prep")
                    nc.tensor.matmul(
                        prep[0:64, :], lhsT=ones16[srow:srow + 1, :],
                        rhs=s16[srow:srow + 1, :], start=True, stop=True)
                    rsb = rpool.tile([P, 512], F32, tag="rsb")
                    nc.vector.reciprocal_approx_fast(rsb[0:64, :],
                                                     prep[0:64, :])
                    crows = ds(64, 64) if odd else ds(0, 64)
                    nc.vector.tensor_tensor(
                        ctxT[base:base + 64, s, hp, :], pc[crows, :],
                        rsb[0:64, :], ALU.mult)
        sBC.close()  # qT/kT/vv2 dead

        # ---------------- Stage D: out-proj + residual -> o1 (SBUF) -------
        pDF = sDF.enter_context(tc.tile_pool(name="pDF", bufs=1, side="right"))
        o1 = pDF.tile([P, 2, NTO, D], F32, tag="o1")
        ctv = ctxT.rearrange("p s (a b) t -> p s a b t", b=2)
        uo = IWS if FP8_O else None
        with ExitStack() as sD:
            xrp = sD.enter_context(tc.tile_pool(name="Dx", bufs=3))
            pp2 = sD.enter_context(tc.tile_pool(name="Dp", bufs=3, space="PSUM"))
            for s in range(2):
                for tt in range(NTO):
                    ps = pp2.tile([P, 1024], F32, tag="pp2")
                    for mc in range(2):
                        mm_win(ps[:, ds(mc * 512, 512)],
                               lambda o2: ctv[:, s, o2, :, ts(tt, P)],
                               lambda o2: wo_sb[:, o2, :, ds(mc * 512, 512)],
                               4, FP8_O, skip=True)
                    xr = xrp.tile([P, D], F32, tag="xr")
                    nc.sync.dma_start(xr, xs[s, ds(128 + tt * P, P), :])
                    dst = o1[:, s, tt, :]
                    if uo is not None:
                        if tt % 2 == 0:
                            nc.vector.tensor_scalar(dst, ps, scalar1=uo,
                                                    scalar2=None, op0=ALU.mult)
                        else:
                            nc.scalar.activation(dst, ps, AF.Copy, bias=0.0,
                                                 scale=uo)
                        nc.gpsimd.tensor_tensor(dst, dst, xr, ALU.add)
                    else:
                        nc.vector.tensor_tensor(dst, ps, xr, ALU.add)
                    if hbo:
                        nc.any.tensor_tensor(dst, dst, bias_sb["bo"], ALU.add)
        sCD.close()  # ctxT dead

        # ---------------- Stage E: LN2 -> z2T ----------------
        z2T = pDF.tile([P, 2, NBD, TOWN], dt_f1, tag="z2T")
        with ExitStack() as sE:
            stp2 = sE.enter_context(tc.tile_pool(name="Est", bufs=4))
            zp2 = sE.enter_context(tc.tile_pool(name="Ez", bufs=2))
            tps2 = sE.enter_context(tc.tile_pool(name="Etp", bufs=3, space="PSUM"))
            for s in range(2):
                for tt in range(NTO):
                    ln_body(stp2, zp2, tps2, o1[:, s, tt, :], z2T[:, s],
                            ts(tt, P), s * NTO + tt)

        # ---------------- Stage F: FFN ----------------
        h1T = pDF.tile([P, 2, NBH, TOWN], dt_f2, tag="h1T")
        z2v = z2T.rearrange("p s (a b) t -> p s a b t", b=2)
        u1 = IWS if FP8_F1 else 1.0
        with ExitStack() as sF:
            fp = sF.enter_context(tc.tile_pool(name="Fp", bufs=3, space="PSUM"))
            fw = sF.enter_context(tc.tile_pool(name="Fw", bufs=2))
            # fc1 (+gelu): two nb blocks per 2-bank psum, one gelu drain
            if FP8_F1:
                for s in range(2):
                    for nbg in range(NBH // 2):
                        ps = fp.tile([P, 1024], F32, tag="f_pp")
                        for nbl in range(2):
                            nb = nbg * 2 + nbl
                            mm_win(ps[:, ds(nbl * 512, 512)],
                                   lambda o2: w1_sb[:, o2, :, ts(nb, P)],
                                   lambda o2: z2v[:, s, o2, :, :],
                                   4, True, skip=True)
                        if hb1:
                            for nbl in range(2):
                                nb = nbg * 2 + nbl
                                nc.scalar.activation(
                                    h1T[:, s, nb, :], ps[:, ds(nbl * 512, 512)],
                                    AF.Gelu, bias=bias_sb["b1"][:, nb:nb + 1],
                                    scale=u1)
                        else:
                            nc.scalar.activation(
                                h1T[:, s, ds(nbg * 2, 2), :].rearrange(
                                    "p a t -> p (a t)"),
                                ps, AF.Gelu, bias=0.0, scale=u1)
            else:
                # fp16 fc1: stream w1 in four 1024-col slabs
                for nch in range(4):
                    wsb = fw.tile([P, 4, 2, 1024], F16, tag="w1slab")
                    nc.sync.dma_start(wsb, w1[:, :, :, ds(nch * 1024, 1024)])
                    for s in range(2):
                        for nbgl in range(4):
                            ps = fp.tile([P, 1024], F32, tag="f_pp")
                            for nbl in range(2):
                                nbw = nbgl * 2 + nbl
                                mm_win(ps[:, ds(nbl * 512, 512)],
                                       lambda o2: wsb[:, o2, :, ts(nbw, P)],
                                       lambda o2: z2v[:, s, o2, :, :],
                                       4, False, skip=True)
                            nb = nch * 8 + nbgl * 2
                            if hb1:
                                for nbl in range(2):
                                    nc.scalar.activation(
                                        h1T[:, s, nb + nbl, :],
                                        ps[:, ds(nbl * 512, 512)], AF.Gelu,
                                        bias=bias_sb["b1"][:, nb + nbl:
                                                           nb + nbl + 1],
                                        scale=u1)
                            else:
                                nc.scalar.activation(
                                    h1T[:, s, ds(nb, 2), :].rearrange(
                                        "p a t -> p (a t)"),
                                    ps, AF.Gelu, bias=0.0, scale=u1)

            # fc2 + residual -> ys
            h1v = h1T.rearrange("p s (a b) t -> p s a b t", b=2)
            u2 = IWS if FP8_F2 else None
            fyp = sF.enter_context(tc.tile_pool(name="Fy", bufs=3))
            if FP8_F2:
                for s in range(2):
                    for tt in range(NTO):
                        ps = fp.tile([P, 1024], F32, tag="f_pp")
                        for mc in range(2):
                            mm_win(ps[:, ds(mc * 512, 512)],
                                   lambda o2: h1v[:, s, o2, :, ts(tt, P)],
                                   lambda o2: w2_sb[:, o2, :, ds(mc * 512, 512)],
                                   16, True, skip=True)
                        yt = fyp.tile([P, D], F32, tag="yt")
                        if tt % 2 == 0:
                            nc.vector.tensor_scalar(yt, ps, scalar1=u2,
                                                    scalar2=None, op0=ALU.mult)
                        else:
                            nc.scalar.activation(yt, ps, AF.Copy, bias=0.0,
                                                 scale=u2)
                        nc.gpsimd.tensor_tensor(yt, yt, o1[:, s, tt, :],
                                                ALU.add)
                        if hb2:
                            nc.any.tensor_tensor(yt, yt, bias_sb["b2"], ALU.add)
                        nc.sync.dma_start(ys[s, ts(tt, P), :], yt)
            else:
                # fp16 fc2: stream w2 in two 512-col slabs
                for mc in range(2):
                    wsb2 = fw.tile([P, 16, 2, 512], F16, tag="w2slab")
                    nc.sync.dma_start(wsb2, w2[:, :, :, ds(mc * 512, 512)])
                    for s in range(2):
                        for tt in range(NTO):
                            ps = fp.tile([P, 1024], F32, tag="f_pp")
                            mm_win(ps[:, 0:512],
                                   lambda o2: h1v[:, s, o2, :, ts(tt, P)],
                                   lambda o2: wsb2[:, o2, :, :],
                                   16, False, skip=True)
                            yt = fyp.tile([P, 512], F32, tag="yt2")
                            o1r = o1[:, s, tt, ds(mc * 512, 512)]
                            nc.vector.tensor_copy(yt, ps[:, 0:512])
                            nc.gpsimd.tensor_tensor(yt, yt, o1r, ALU.add)
                            if hb2:
                                nc.any.tensor_tensor(
                                    yt, yt, bias_sb["b2"][:, ds(mc * 512, 512)],
                                    ALU.add)
                            nc.sync.dma_start(
                                ys[s, ts(tt, P), ds(mc * 512, 512)], yt)
        sDF.close()
        sW.close()
        sConst.close()

    nc.compile()
    return nc


_PROGRAM_CACHE = {}


def get_program(has_bias):
    key = tuple(has_bias)
    if key not in _PROGRAM_CACHE:
        nc = bacc.Bacc("TRN2", target_bir_lowering=False, debug=False,
                       num_devices=NCORES)
        _PROGRAM_CACHE[key] = _emit(nc, tuple(has_bias))
    return _PROGRAM_CACHE[key]


def _wlayout(w, ko, fp8):
    """[Kdim, N] -> [P, ko, 2, N] (contraction k = o2*256 + kt*128 + p)."""
    kdim, n = w.shape
    assert kdim == ko * 256
    wr = w.reshape(ko, 2, P, n).transpose(2, 0, 1, 3)
    wr = np.ascontiguousarray(wr)
    if fp8:
        return (wr * WS).astype(ml_dtypes.float8_e4m3)
    return wr.astype(np.float16)


def make_host_inputs(x, ln1_g, ln1_b, wq, wk, wv, bq, bk, bv, wo, bo,
                     ln2_g, ln2_b, w1, b1, w2, b2):
    """Fold LN affine params into weights, build per-core sharded inputs.
    Returns (in_maps, has_bias)."""
    f = np.float32
    x = np.asarray(x, f)
    wq_f = np.asarray(wq, f) * np.asarray(ln1_g, f)[:, None]
    wk_f = np.asarray(wk, f) * np.asarray(ln1_g, f)[:, None]
    wv_f = np.asarray(wv, f) * np.asarray(ln1_g, f)[:, None]
    w1_f = np.asarray(w1, f) * np.asarray(ln2_g, f)[:, None]
    bq_f = (np.asarray(bq, f) + np.asarray(ln1_b, f) @ np.asarray(wq, f)).astype(f)
    bk_f = (np.asarray(bk, f) + np.asarray(ln1_b, f) @ np.asarray(wk, f)).astype(f)
    bv_f = (np.asarray(bv, f) + np.asarray(ln1_b, f) @ np.asarray(wv, f)).astype(f)
    b1_f = (np.asarray(b1, f) + np.asarray(ln2_b, f) @ np.asarray(w1, f)).astype(f)
    bo_f = np.asarray(bo, f)
    b2_f = np.asarray(b2, f)
    wo_f = np.asarray(wo, f)
    w2_f = np.asarray(w2, f)

    has_bias = tuple(bool(np.any(v)) for v in (bq_f, bk_f, bv_f, bo_f, b1_f, b2_f))
    hbq, hbk, hbv, hbo, hb1, hb2 = has_bias

    wq_h = _wlayout(wq_f, 4, FP8_QKV)
    wk_h = _wlayout(wk_f, 4, FP8_QKV)
    wv_h = _wlayout(wv_f, 4, FP8_QKV)
    wo_h = _wlayout(wo_f, 4, FP8_O)
    w1_h = _wlayout(w1_f, 4, FP8_F1)
    w2_h = _wlayout(w2_f, 16, FP8_F2)

    r = np.arange(P)[:, None]    # k within block
    c = np.arange(256)[None, :]  # q within window
    band = ((c >= r) & (c <= r + 128)).astype(np.float16)
    m0 = band[:, 128:256]        # j=0: lower-tri (c <= r)
    m4 = band[:, 0:128]
    maskC_h = np.concatenate([m0, band, band, band, m4], axis=1)
    maskC_0 = maskC_h.copy()
    maskC_0[:, 0:128] = 0.0      # first chunk: halo block invalid
    ident = np.eye(P, dtype=np.float16)

    in_maps = []
    for core in range(NCORES):
        b, cchunk = divmod(core, NCORES // B)
        start = cchunk * CHUNK - HALO
        xsh = np.zeros((2, TL, D), f)
        for s in range(2):
            lo = start + s
            idx = lo + 2 * np.arange(TL)
            valid = idx >= 0
            xsh[s, valid] = x[b, idx[valid]]
        m = {
            "xs": xsh, "wq": wq_h, "wk": wk_h, "wv": wv_h, "wo": wo_h,
            "w1": w1_h, "w2": w2_h,
            "maskC": maskC_0 if cchunk == 0 else maskC_h,
            "ident": ident,
        }
        if hbq:
            m["bq"] = bq_f.reshape(NBD, P)
        if hbk:
            m["bk"] = bk_f.reshape(NBD, P)
        if hbv:
            m["bv"] = bv_f
        if hbo:
            m["bo"] = bo_f
        if hb1:
            m["b1"] = b1_f.reshape(NBH, P)
        if hb2:
            m["b2"] = b2_f
        in_maps.append(m)
    return in_maps, has_bias


def assemble_output(core_outs):
    """core_outs: list of 8 arrays [2, 512, D] -> full [B, L, D]."""
    out = np.empty((B, L, D), np.float32)
    for core, ysh in enumerate(core_outs):
        b, c = divmod(core, NCORES // B)
        for s in range(2):
            out[b, c * CHUNK + s: (c + 1) * CHUNK: 2, :] = ysh[s]
    return out


def run(inputs, trace=False):
    in_maps, has_bias = make_host_inputs(**inputs)
    nc = get_program(has_bias)
    from concourse.bass_utils import run_bass_kernel_spmd
    res = run_bass_kernel_spmd(nc, in_maps, core_ids=list(range(NCORES)),
                               trace=trace)
    out = assemble_output([r["ys"] for r in res.results])
    return out, res


def kernel(**inputs):
    out, _ = run(inputs, trace=False)
    return out
